# revision 1
# baseline (speedup 1.0000x reference)
"""Trainium2 Bass kernel for nn_NodeNet: GNN message passing + 12-qubit TTN circuit.

Math: the reference's statevector circuit contracts exactly to per-node
Bloch-vector chains (every CNOT block keeps only its target wire; the
measurement is <Z_9>; the circuit is a tree so alive wires stay in
product states). Per node the whole circuit is ~60 scalar ops.

Sharding: E-parallel over the 8 cores. Core k owns edge columns
Ek = [1024k, 1024k+1024):
  bo_k^T[d,e] = sum_n X[n,d] Ro[n,e]      (local, contraction over nodes)
  beo_k[e,d]  = e[e] * bo_k[e,d]
  partial mi^T[d,n] = sum_{e in Ek} beo[e,d] RiT[e,n]
ReduceScatter sums the partials over cores and hands core k its own
128-node slice, which feeds the Bloch-chain circuit; per-core outputs
are concatenated on the host.

Precision: the relation matrices are 0/1-valued, so bf16 is exact and
halves both DMA bytes and PE time (fp32 matmul streams at 1/4 rate).
X and beo are carried as bf16 high+low splits packed side by side in
the stationary operand (M=8), recovering fp32-grade accuracy with the
same matmul count; the split halves are summed during PSUM eviction.

Both layouts of each relation shard (natural [n,e] and transposed
[e,n]) are passed from the host: zero on-chip 128x128 transposes at
the cost of 2x matrix DMA - measured faster, the PE is otherwise the
phase-1 bottleneck.
"""

import ml_dtypes
import numpy as np

import bass_rust
import concourse.bass as bass
import concourse.mybir as mybir
import concourse.tile as tile
from concourse.bass_utils import run_bass_kernel_spmd
from concourse.masks import make_identity

F32 = mybir.dt.float32
BF16 = mybir.dt.bfloat16
N_CORES = 8
N, E, D = 1024, 8192, 4
ES = E // N_CORES        # 1024 edge columns per core
P = 128                  # partitions / nodes per core
NCH = N // P             # 8 node chunks
ECH = ES // P            # 8 edge chunks per core
MW = 36                  # stationary width: high split at 0:4, low at 32:36
LO = 32                  # (PSUM partition reads must be 32-aligned)

_BLOCKS = [(0, 1, (0, 1)), (2, 3, (3, 2)), (4, 5, (4, 5)), (6, 7, (7, 6)),
           (8, 9, (8, 9)), (10, 11, (11, 10)), (1, 2, (1, 2)), (5, 6, (6, 5)),
           (9, 10, (10, 9)), (2, 5, (2, 5)), (5, 9, (5, 9))]

# ---------------------------------------------------------------------------
# Column layout of the M-angle tile (device) and ar (ReduceScatter) rows
# ---------------------------------------------------------------------------
# M cols 0:6  = layer-A target wires  [w1, w6, w10, w2, w5, w9]
# M cols 6:12 = layer-A control wires [w0, w7, w11, w3, w4, w8] (block-paired)
# Sources: wire w<4 -> mi[:,w]; 4<=w<8 -> mo[:,w-4]; w>=8 -> X[:,w-8]
#   mi lands at cols {0,3,6,9} (stride 3): order [mi1, mi2, mi0, mi3]
#   mo lands at cols {1,4,7,10}: order [mo2, mo1, mo3, mo0]
#   X  lands at cols {2,5,8,11}: order [X2, X1, X3, X0]
# ar rows = post-transpose mm cols = [mi1, mi2, mi0, mi3, mo2, mo1, mo3, mo0]
A_BLOCKS = [0, 3, 5, 1, 2, 4]     # block idx per A-target col
B_BLOCKS = [6, 7, 8]              # b-cols [w2, w5, w9] <- a-cols [w1, w6, w10]
XK_PERM = [2, 1, 3, 0]            # X columns in M stride-3 order

# ---------------------------------------------------------------------------
# Host-side circuit-constant preparation
# ---------------------------------------------------------------------------

_PAULI = np.array([
    [[0, 1], [1, 0]],
    [[0, -1j], [1j, 0]],
    [[1, 0], [0, -1]],
], dtype=np.complex128)


def _rot_so3(p):
    """SO(3) Bloch rotation of Rot(phi, theta, omega) = RZ(om) RY(th) RZ(phi)."""
    phi, th, om = float(p[0]), float(p[1]), float(p[2])
    c, s = np.cos(th / 2), np.sin(th / 2)
    U = np.array([
        [np.exp(-0.5j * (phi + om)) * c, -np.exp(0.5j * (phi - om)) * s],
        [np.exp(-0.5j * (phi - om)) * s, np.exp(0.5j * (phi + om)) * c],
    ])
    R = np.empty((3, 3))
    for i in range(3):
        for j in range(3):
            R[i, j] = 0.5 * np.real(
                np.trace(_PAULI[i] @ U @ _PAULI[j] @ U.conj().T))
    return R


# circuit-constants column layout (offsets into the ck segment of smalls)
CK_AT = 0        # layer A target rot entries T[i][j2], j2 in {0,2}
CK_AC = 36       # layer A control row2 entries C2[j2]
CK_BT = 48       # layer B target entries T[i][j]
CK_BC = 75       # layer B control row2
CK_C19 = 84      # R19 full 3x3 (block 9 target rot)
CK_C18 = 93      # R18 row 2 (block 9 control rot)
CK_C21 = 96      # R21 row 2 (block 10 target rot)
CK_C20 = 99      # R20 row 2 (block 10 control rot)
CK_W = 102

# smalls tensor layout: [xk_perm(4) | eperm(ECH) | ck(CK_W)]
SM_XK = 0
SM_EP = 4
SM_CK = 4 + ECH
SM_W = SM_CK + CK_W


def _pack_ck(theta):
    th = np.asarray(theta, np.float64)
    R = [_rot_so3(th[3 * k:3 * k + 3]) for k in range(23)]
    ck = np.zeros(CK_W, np.float64)

    for t, bidx in enumerate(A_BLOCKS):
        w1, w2, (c, tt) = _BLOCKS[bidx]
        k1, k2 = 2 * bidx, 2 * bidx + 1
        Rc = R[k1] if c == w1 else R[k2]
        Rt = R[k1] if tt == w1 else R[k2]
        for i in range(3):
            for jj, j2 in enumerate((0, 2)):
                ck[CK_AT + (i * 2 + jj) * 6 + t] = Rt[i, j2]
        for jj, j2 in enumerate((0, 2)):
            ck[CK_AC + jj * 6 + t] = Rc[2, j2]

    for t, bidx in enumerate(B_BLOCKS):
        w1, w2, (c, tt) = _BLOCKS[bidx]
        k1, k2 = 2 * bidx, 2 * bidx + 1
        Rc = R[k1] if c == w1 else R[k2]
        Rt = R[k1] if tt == w1 else R[k2]
        for i in range(3):
            for j in range(3):
                ck[CK_BT + (3 * i + j) * 3 + t] = Rt[i, j]
        for j in range(3):
            ck[CK_BC + j * 3 + t] = Rc[2, j]

    # layer C: block 9 = (2,5,(2,5)): control rot R[18] (wire2), target R[19]
    #          block 10 = (5,9,(5,9)): control rot R[20] (wire5), target R[21]
    ck[CK_C19:CK_C19 + 9] = R[19].reshape(-1)
    ck[CK_C18:CK_C18 + 3] = R[18][2]
    ck[CK_C21:CK_C21 + 3] = R[21][2]
    ck[CK_C20:CK_C20 + 3] = R[20][2]
    return ck.astype(np.float32)


# ---------------------------------------------------------------------------
# Walrus workaround: this build rejects >1 sync-wait per instruction
# ---------------------------------------------------------------------------


def _split_multi_waits(nc):
    for f in nc.m.functions:
        for bb in f.blocks:
            out = []
            for inst in bb.instructions:
                si = inst.sync_info
                if si is not None and si.on_wait and len(si.on_wait) > 1:
                    waits = list(si.on_wait)
                    for i, w in enumerate(waits[:-1]):
                        out.append(mybir.InstNoOp(
                            name=f"{inst.name}_wsplit{i}",
                            engine=inst.engine,
                            ins=[], outs=[],
                            sync_info=bass_rust.SyncInfo(
                                on_wait=[w], on_update=[]),
                        ))
                    inst.sync_info = bass_rust.SyncInfo(
                        on_wait=[waits[-1]], on_update=list(si.on_update))
                out.append(inst)
            bb.instructions = out


# ---------------------------------------------------------------------------
# Device kernel
# ---------------------------------------------------------------------------


def _build_nc():
    nc = bass.Bass("TRN2", target_bir_lowering=False, num_devices=N_CORES)

    ro_nat = nc.declare_dram_parameter("ro_nat", [N, ES], BF16, isOutput=False)
    ri_nat = nc.declare_dram_parameter("ri_nat", [N, ES], BF16, isOutput=False)
    rot_t = nc.declare_dram_parameter("rot_t", [ES, N], BF16, isOutput=False)
    rit_t = nc.declare_dram_parameter("rit_t", [ES, N], BF16, isOutput=False)
    xsp_d = nc.declare_dram_parameter("xsp", [P, NCH * MW], BF16,
                                      isOutput=False)
    smalls = nc.declare_dram_parameter("smalls", [P, SM_W], F32,
                                       isOutput=False)
    out = nc.declare_dram_parameter("out", [P, 1], F32, isOutput=True)

    HPI = float(np.pi / 2)
    PI = float(np.pi)
    MUL = mybir.AluOpType.mult
    ADD = mybir.AluOpType.add

    with tile.TileContext(nc) as tc:
        with (
            tc.tile_pool(name="big", bufs=1) as big,
            tc.tile_pool(name="small", bufs=1) as small,
            tc.tile_pool(name="work", bufs=1) as work,
            tc.tile_pool(name="acc", bufs=2, space="PSUM") as accp,
            tc.tile_pool(name="tbp", bufs=2, space="PSUM") as tbp,
            tc.tile_pool(name="dram", bufs=1, space="DRAM") as dram,
        ):
            # ---- small inputs: two DMAs ----------------------------------
            xsp_sb = small.tile([P, NCH * MW], BF16, name="xsp_sb")
            nc.sync.dma_start(xsp_sb[:], xsp_d[:])
            sm_sb = small.tile([P, SM_W], F32, name="sm_sb")
            nc.sync.dma_start(sm_sb[:], smalls[:])

            def ckc(off, n=1):
                return sm_sb[:, SM_CK + off:SM_CK + off + n]

            # preload the ACT Sin table set while DMAs stream
            warm = small.tile([P, 1], F32, name="warm")
            nc.vector.memset(warm[:], 0.0)
            nc.scalar.activation(warm[:], warm[:],
                                 mybir.ActivationFunctionType.Sin)

            ident = small.tile([P, P], F32, name="ident")
            make_identity(nc, ident)

            # ---- big matrix shards, all resident -------------------------
            nat_sb = {}   # nat_sb[rel][nchunk]: [128 nodes, ES] bf16
            tt_sb = {}    # tt_sb[rel][echunk]: [128 edges, N] bf16
            for rel, src in (("o", ro_nat), ("i", ri_nat)):
                nat_sb[rel] = []
                for c in range(NCH):
                    t = big.tile([P, ES], BF16, name=f"nat_{rel}{c}",
                                 tag=f"nat_{rel}{c}")
                    nc.sync.dma_start(t[:], src[c * P:(c + 1) * P, :])
                    nat_sb[rel].append(t)
            for rel, src in (("i", rit_t), ("o", rot_t)):
                tt_sb[rel] = []
                for c in range(ECH):
                    t = big.tile([P, N], BF16, name=f"tt_{rel}{c}",
                                 tag=f"tt_{rel}{c}")
                    nc.gpsimd.dma_start(t[:], src[c * P:(c + 1) * P, :])
                    tt_sb[rel].append(t)

            # ---- stage 1: bo^T = [Xh|Xl]^T @ Ro, M=8 packed --------------
            # ---- stage 2: transpose-back + e-scale + bf16 split ----------
            beo_sb = {}
            for rel in ("o", "i"):
                boT = work.tile([D, ES], F32, name=f"boT_{rel}",
                                tag=f"boT_{rel}")
                for h in range(2):
                    ps = accp.tile([MW, 512], F32, name=f"boT_ps_{rel}{h}",
                                   tag="acc")
                    for c in range(NCH):
                        nc.tensor.matmul(
                            ps[:],
                            xsp_sb[:, c * MW:(c + 1) * MW],
                            nat_sb[rel][c][:, h * 512:(h + 1) * 512],
                            start=(c == 0), stop=(c == NCH - 1))
                    lo_t = small.tile([D, 512], F32, name=f"lo_b{rel}{h}",
                                      tag="lo_t", bufs=2)
                    nc.scalar.copy(lo_t[:], ps[LO:LO + 4, :])
                    nc.vector.tensor_add(
                        boT[:, h * 512:(h + 1) * 512], ps[0:4, :], lo_t[:])
                beo = work.tile([P, ECH * D], F32, name=f"beo_{rel}",
                                tag=f"beo_{rel}")
                for c in range(ECH):
                    tb = tbp.tile([P, D], F32, name=f"tb_{rel}{c}", tag="tb")
                    nc.tensor.transpose(
                        tb[:], boT[:, c * P:(c + 1) * P], ident[0:D, 0:D])
                    nc.vector.tensor_scalar(
                        beo[:, c * D:(c + 1) * D], tb[:],
                        sm_sb[:, SM_EP + c:SM_EP + c + 1], None, MUL)
                # split into packed [high | low] bf16 (chunk stride MW)
                bhl = work.tile([P, ECH * MW], BF16, name=f"bhl_{rel}",
                                tag=f"bhl_{rel}")
                brs = work.tile([P, ECH * D], F32, name=f"brs_{rel}",
                                tag=f"brs_{rel}")
                nc.vector.memset(bhl[:], 0.0)
                hl4 = bhl.rearrange("p (c m) -> p c m", m=MW)
                hi_view = hl4[:, :, 0:D]
                lo_view = hl4[:, :, LO:LO + D]
                beo3 = beo.rearrange("p (c d) -> p c d", d=D)
                brs3 = brs.rearrange("p (c d) -> p c d", d=D)
                nc.vector.tensor_copy(hi_view, beo3)
                nc.vector.scalar_tensor_tensor(
                    brs3, hi_view, -1.0, beo3, MUL, ADD)
                nc.vector.tensor_copy(lo_view, brs3)
                beo_sb[rel] = bhl

            # ---- stage 3: partial mi^T = [beo_h|beo_l]^T @ RiT, M=8 ------
            # mi pairs beo (from Ro) with RiT; mo pairs bei with RoT.
            # ar rows: [mi1, mi2, mi0, mi3, mo2, mo1, mo3, mo0]
            ar_in = dram.tile([NCH, 8, P], F32, name="ar_in")
            ar_out = dram.tile([8, P], F32, name="ar_out")
            for ri, (rel_b, rel_t) in enumerate((("o", "i"), ("i", "o"))):
                miT = work.tile([D, N], F32, name=f"miT_{ri}", tag=f"miT_{ri}")
                for h in range(2):
                    ps = accp.tile([MW, 512], F32, name=f"miT_ps_{ri}{h}",
                                   tag="acc")
                    for c in range(ECH):
                        nc.tensor.matmul(
                            ps[:],
                            beo_sb[rel_b][:, c * MW:(c + 1) * MW],
                            tt_sb[rel_t][c][:, h * 512:(h + 1) * 512],
                            start=(c == 0), stop=(c == ECH - 1))
                    lo_t = small.tile([D, 512], F32, name=f"lo_m{ri}{h}",
                                      tag="lo_t", bufs=2)
                    nc.scalar.copy(lo_t[:], ps[LO:LO + 4, :])
                    nc.vector.tensor_add(
                        miT[:, h * 512:(h + 1) * 512], ps[0:4, :], lo_t[:])
                miT3 = miT.rearrange("d (c p) -> d c p", p=P)
                if ri == 0:
                    # mi rows [1,2] -> ar[0:2]; rows [0,3] -> ar[2:4]
                    nc.sync.dma_start(
                        ar_in[:, 0:2].rearrange("c r p -> r c p"), miT3[1:3])
                    nc.sync.dma_start(
                        ar_in[:, 2:4].rearrange("c r p -> r c p"),
                        miT3[0:4:3])
                else:
                    # mo rows [2,1,3,0] -> ar[4:8]
                    for slot, row in ((4, 2), (5, 1), (6, 3), (7, 0)):
                        nc.sync.dma_start(
                            ar_in[:, slot:slot + 1].rearrange(
                                "c r p -> r c p"),
                            miT3[row:row + 1])

            nc.gpsimd.collective_compute(
                "ReduceScatter",
                mybir.AluOpType.add,
                replica_groups=[list(range(N_CORES))],
                ins=[ar_in.opt()],
                outs=[ar_out.opt()],
            )

            # ---- circuit: build M angles ---------------------------------
            mmT = small.tile([8, P], F32, name="mmT")
            nc.sync.dma_start(mmT[:], ar_out[:])
            mm_ps = tbp.tile([P, 8], F32, name="mm_ps", tag="mm")
            nc.tensor.transpose(mm_ps[:], mmT[:], ident[0:8, 0:8])

            # cols 0:12 = m (stride-3 interleave), cols 12:24 = m + pi/2
            m_ang = small.tile([P, 24], F32, name="m_ang")
            m3 = m_ang.rearrange("p (c t) -> p c t", t=3)
            nc.vector.tensor_copy(m3[:, 0:4, 0], mm_ps[:, 0:4])
            nc.vector.tensor_copy(m3[:, 0:4, 1], mm_ps[:, 4:8])
            nc.vector.tensor_copy(m3[:, 0:4, 2], sm_sb[:, SM_XK:SM_XK + 4])
            nc.vector.tensor_scalar(
                m_ang[:, 12:24], m_ang[:, 0:12], HPI, None, ADD)

            # range-reduce into [-pi, pi]: m2 = clamp(m - 2pi*rne(m/2pi));
            # f32->i32 cast is round-to-nearest-even on the DVE (HW-checked)
            TWO_PI = float(2 * np.pi)
            t_f = small.tile([P, 24], F32, name="t_f")
            t_i = small.tile([P, 24], mybir.dt.int32, name="t_i")
            t_r = small.tile([P, 24], F32, name="t_r")
            m2 = small.tile([P, 24], F32, name="m2")
            nc.vector.tensor_scalar(
                t_f[:], m_ang[:], float(1.0 / TWO_PI), None, MUL)
            nc.vector.tensor_copy(t_i[:], t_f[:])
            nc.vector.tensor_copy(t_r[:], t_i[:])
            nc.vector.scalar_tensor_tensor(
                m2[:], t_r[:], -TWO_PI, m_ang[:], MUL, ADD)
            nc.vector.tensor_scalar(
                m2[:], m2[:], PI, -PI,
                mybir.AluOpType.min, mybir.AluOpType.max)
            sxz = small.tile([P, 24], F32, name="sxz")
            nc.scalar.activation(sxz[:], m2[:],
                                 mybir.ActivationFunctionType.Sin)

            TT = nc.vector.tensor_tensor
            STT = nc.vector.scalar_tensor_tensor

            # ---- layer A: 6 blocks vectorized [128, 6] -------------------
            sxa, sza = sxz[:, 6:12], sxz[:, 18:24]
            sxb, szb = sxz[:, 0:6], sxz[:, 12:18]
            az6 = small.tile([P, 6], F32, name="az6")
            tmp6 = small.tile([P, 6], F32, name="tmp6")
            TT(az6[:], ckc(CK_AC, 6), sxa, MUL)
            TT(tmp6[:], ckc(CK_AC + 6, 6), sza, MUL)
            TT(az6[:], az6[:], tmp6[:], ADD)

            abx = small.tile([P, 6], F32, name="abx")
            aby = small.tile([P, 6], F32, name="aby")
            abz = small.tile([P, 6], F32, name="abz")
            for i, dst in enumerate((abx, aby, abz)):
                TT(dst[:], ckc(CK_AT + (i * 2) * 6, 6), sxb, MUL)
                TT(tmp6[:], ckc(CK_AT + (i * 2 + 1) * 6, 6), szb, MUL)
                TT(dst[:], dst[:], tmp6[:], ADD)
            TT(aby[:], az6[:], aby[:], MUL)
            TT(abz[:], az6[:], abz[:], MUL)

            # ---- layer B: 3 blocks vectorized [128, 3] -------------------
            # a-cols 0:3 (w1, w6, w10), b-cols 3:6 (w2, w5, w9) - contiguous
            av = [t[:, 0:3] for t in (abx, aby, abz)]
            bv = [t[:, 3:6] for t in (abx, aby, abz)]
            az3 = small.tile([P, 3], F32, name="az3")
            tmp3 = small.tile([P, 3], F32, name="tmp3")
            TT(az3[:], ckc(CK_BC, 3), av[0], MUL)
            for j in (1, 2):
                TT(tmp3[:], ckc(CK_BC + 3 * j, 3), av[j], MUL)
                TT(az3[:], az3[:], tmp3[:], ADD)
            bbx = small.tile([P, 3], F32, name="bbx")
            bby = small.tile([P, 3], F32, name="bby")
            bbz = small.tile([P, 3], F32, name="bbz")
            for i, dst in enumerate((bbx, bby, bbz)):
                TT(dst[:], ckc(CK_BT + (3 * i) * 3, 3), bv[0], MUL)
                for j in (1, 2):
                    TT(tmp3[:], ckc(CK_BT + (3 * i + j) * 3, 3), bv[j], MUL)
                    TT(dst[:], dst[:], tmp3[:], ADD)
            TT(bby[:], az3[:], bby[:], MUL)
            TT(bbz[:], az3[:], bbz[:], MUL)

            # ---- layer C: blocks 9 then 10, [128, 1] ---------------------
            # cols of bb*: 0 = w2, 1 = w5, 2 = w9
            def col(t, j):
                return t[:, j:j + 1]

            s9 = small.tile([P, 1], F32, name="s9")
            u = small.tile([P, 1], F32, name="u")
            nc.vector.tensor_scalar(s9[:], col(bbx, 0), ckc(CK_C18), None, MUL)
            STT(s9[:], col(bby, 0), ckc(CK_C18 + 1), s9[:], MUL, ADD)
            STT(s9[:], col(bbz, 0), ckc(CK_C18 + 2), s9[:], MUL, ADD)

            w5 = [small.tile([P, 1], F32, name=f"w5{i}") for i in range(3)]
            for i in range(3):
                nc.vector.tensor_scalar(
                    w5[i][:], col(bbx, 1), ckc(CK_C19 + 3 * i), None, MUL)
                STT(w5[i][:], col(bby, 1), ckc(CK_C19 + 3 * i + 1),
                    w5[i][:], MUL, ADD)
                STT(w5[i][:], col(bbz, 1), ckc(CK_C19 + 3 * i + 2),
                    w5[i][:], MUL, ADD)
            TT(w5[1][:], s9[:], w5[1][:], MUL)
            TT(w5[2][:], s9[:], w5[2][:], MUL)

            s10 = small.tile([P, 1], F32, name="s10")
            nc.vector.tensor_scalar(s10[:], w5[0][:], ckc(CK_C20), None, MUL)
            STT(s10[:], w5[1][:], ckc(CK_C20 + 1), s10[:], MUL, ADD)
            STT(s10[:], w5[2][:], ckc(CK_C20 + 2), s10[:], MUL, ADD)

            nc.vector.tensor_scalar(u[:], col(bbx, 2), ckc(CK_C21), None, MUL)
            STT(u[:], col(bby, 2), ckc(CK_C21 + 1), u[:], MUL, ADD)
            STT(u[:], col(bbz, 2), ckc(CK_C21 + 2), u[:], MUL, ADD)

            zf = small.tile([P, 1], F32, name="zf")
            TT(zf[:], s10[:], u[:], MUL)
            res = small.tile([P, 1], F32, name="res")
            nc.vector.tensor_scalar(res[:], zf[:], -PI, PI, MUL, ADD)
            nc.sync.dma_start(out[:], res[:])

    return nc


_NC_CACHE = {}
_RUN_KWARGS = {}      # test harness can set e.g. {"trace": True}
_LAST_RESULTS = []    # BassKernelResults of the most recent run


def _get_nc():
    if "nc" not in _NC_CACHE:
        nc = _build_nc()
        _split_multi_waits(nc)
        _NC_CACHE["nc"] = nc
    return _NC_CACHE["nc"]


def _host_prep_x(X):
    """xsp[p, c*MW + {0:4,LO:LO+4}] = {high,low} split of X[c*128+p, :]."""
    bf = ml_dtypes.bfloat16
    xh = X.astype(bf).astype(np.float32)
    xl = X - xh
    xsp = np.zeros((P, NCH, MW), np.float32)
    xsp[:, :, 0:D] = xh.reshape(NCH, P, D).transpose(1, 0, 2)
    xsp[:, :, LO:LO + D] = xl.reshape(NCH, P, D).transpose(1, 0, 2)
    return np.ascontiguousarray(xsp.reshape(P, NCH * MW).astype(bf))


def kernel(X, e, Ri, Ro, theta):
    X = np.ascontiguousarray(np.asarray(X, np.float32))
    e = np.ascontiguousarray(np.asarray(e, np.float32))
    Ri = np.asarray(Ri, np.float32)
    Ro = np.asarray(Ro, np.float32)
    theta = np.asarray(theta, np.float32)

    bf = ml_dtypes.bfloat16
    xsp = _host_prep_x(X)
    ck1 = _pack_ck(theta)

    in_maps = []
    for k in range(N_CORES):
        ek = slice(k * ES, (k + 1) * ES)
        sm = np.empty((P, SM_W), np.float32)
        sm[:, SM_XK:SM_XK + 4] = X[k * P:(k + 1) * P][:, XK_PERM]
        sm[:, SM_EP:SM_EP + ECH] = e[ek].reshape(ECH, P).T
        sm[:, SM_CK:] = ck1[None, :]
        in_maps.append({
            "ro_nat": np.ascontiguousarray(Ro[:, ek].astype(bf)),
            "ri_nat": np.ascontiguousarray(Ri[:, ek].astype(bf)),
            "rot_t": np.ascontiguousarray(Ro[:, ek].T.astype(bf)),
            "rit_t": np.ascontiguousarray(Ri[:, ek].T.astype(bf)),
            "xsp": xsp,
            "smalls": np.ascontiguousarray(sm),
        })

    nc = _get_nc()
    res = run_bass_kernel_spmd(nc, in_maps, core_ids=list(range(N_CORES)),
                               **_RUN_KWARGS)
    _LAST_RESULTS.clear()
    _LAST_RESULTS.append(res)
    return np.concatenate(
        [res.results[k]["out"].reshape(-1) for k in range(N_CORES)]
    ).astype(np.float32)



# revision 20
# speedup vs baseline: 2.9726x; 2.9726x over previous
"""Trainium2 Bass kernel for nn_NodeNet: GNN message passing + 12-qubit TTN.

Collective-free sharding: the host owns the edge partition, so core k
receives exactly the edges whose TARGET node lands in its 128-node
slice - once for mi (targets = idx_i) and once for mo (targets =
idx_o). Both contractions are then fully local:

  stage 1 (gather):  beo[j] = e_j * X[src_j]   as a matmul against the
      one-hot source matrix, chunked 128x128 so each chunk's stationary
      is the fp8 0/1 block (exact) and the moving operand is X hi/lo
      bf16 [128, 8]. Edges are sorted by source chunk; per-chunk runs
      are padded to RPAD=256 (max real run ~170).
  stage 2: evict hi+lo, scale by e (f32), re-split to bf16 hi/lo.
  stage 3 (scatter): mi[n] = sum_j beo[j] * RiT[j, n] with stationary =
      the one-hot target block [128, 128] fp8 and moving = beo hi/lo
      [128, 8]; PSUM accumulates over the 16 j-blocks and the output is
      node-partitioned [128, 8] directly - no transposes, no collective.

The circuit contracts to per-node Bloch chains (CNOT target keeps
(x, az*y, az*z); measurement is <Z_9>). Every linear term collapses to
A*sin(m + phi) with host-precomputed amplitude/phase, so the whole
chain is ~30 wide vector ops: build 30 angle columns, wrap to [-pi,pi],
one Sin activation, amplitude multiply, 6 fused (q + az*r) ops, and a
5-op tail.
"""

import ml_dtypes
import numpy as np

import bass_rust
import concourse.bass as bass
import concourse.mybir as mybir
import concourse.tile as tile
from concourse.bass_utils import run_bass_kernel_spmd

F32 = mybir.dt.float32
BF16 = mybir.dt.bfloat16
F8 = mybir.dt.float8e4
N_CORES = 8
N, E, D = 1024, 8192, 4
P = 128                  # partitions / nodes per core
NCH = N // P             # 8 node chunks
RPAD = 256               # padded edges per source chunk (max real ~170)
EPAD = NCH * RPAD        # 2048 padded edges per core per relation
NB = EPAD // P           # 16 j-blocks
PI = float(np.pi)

_SCOPES = False          # test harness can flip on for phase attribution
_FUSED_DVE = False       # use custom-DVE fused ops (add_range_wrap etc.)

_BLOCKS = [(0, 1, (0, 1)), (2, 3, (3, 2)), (4, 5, (4, 5)), (6, 7, (7, 6)),
           (8, 9, (8, 9)), (10, 11, (11, 10)), (1, 2, (1, 2)), (5, 6, (6, 5)),
           (9, 10, (10, 9)), (2, 5, (2, 5)), (5, 9, (5, 9))]

# A-layer blocks 0..5: (target rot idx, ctrl rot idx, target wire, ctrl wire)
A_INFO = []
for _b, (_w1, _w2, (_c, _t)) in enumerate(_BLOCKS[:6]):
    A_INFO.append((2 * _b if _t == _w1 else 2 * _b + 1,
                   2 * _b if _c == _w1 else 2 * _b + 1, _t, _c))

# sin30 layout: 6 groups of [q,r]-pairs over A-blocks [0,3,5,1,2,4]
# (b3 and b2 carry 2 and 4 D's), then 6 ctrl cols.
# D12 = [az6, az7, az7, az8, s9a, s9b, p0a, p0b, p1a, p1b, ua, ub]
_GROUPS = [(0, 2, 0), (2, 4, 3), (6, 2, 5), (8, 4, 1), (12, 8, 2),
           (20, 4, 4)]          # (m30 start, width, A-block)

# smalls column layout
SM_X = 0                 # own-node X angle cols (4)
SM_EA = 4                # e for rel A, replicated x4: [128, 64]
SM_EB = 68
SM_PHI = 132             # phi30
SM_AMP = 162             # amp30
SM_W = 192

# ---------------------------------------------------------------------------
# Host-side circuit-constant preparation
# ---------------------------------------------------------------------------

_PAULI = np.array([
    [[0, 1], [1, 0]],
    [[0, -1j], [1j, 0]],
    [[1, 0], [0, -1]],
], dtype=np.complex128)


def _rot_so3(p):
    """SO(3) Bloch rotation of Rot(phi, theta, omega) = RZ(om) RY(th) RZ(phi)."""
    phi, th, om = float(p[0]), float(p[1]), float(p[2])
    c, s = np.cos(th / 2), np.sin(th / 2)
    U = np.array([
        [np.exp(-0.5j * (phi + om)) * c, -np.exp(0.5j * (phi - om)) * s],
        [np.exp(-0.5j * (phi - om)) * s, np.exp(0.5j * (phi + om)) * c],
    ])
    R = np.empty((3, 3))
    for i in range(3):
        for j in range(3):
            R[i, j] = 0.5 * np.real(
                np.trace(_PAULI[i] @ U @ _PAULI[j] @ U.conj().T))
    return R


def _pack_tables(theta):
    """phi30/amp30 for the amplitude-phase sin tile (see module docstring)."""
    th = np.asarray(theta, np.float64)
    R = [_rot_so3(th[3 * k:3 * k + 3]) for k in range(23)]

    def split_ab(row2, Rt):
        return row2[0] * Rt[0, :], row2[1] * Rt[1, :] + row2[2] * Rt[2, :]

    a_s9, b_s9 = split_ab(R[18][2], R[13])
    v0 = R[20][2, 0] * R[19][0, :]
    v1 = R[20][2, 1] * R[19][1, :] + R[20][2, 2] * R[19][2, :]
    a_p0, b_p0 = split_ab(v0, R[14])
    a_p1, b_p1 = split_ab(v1, R[14])
    a_u, b_u = split_ab(R[21][2], R[16])

    D_order = [(R[12][2], 0), (R[15][2], 3), (R[15][2], 3), (R[17][2], 5),
               (a_s9, 1), (b_s9, 1), (a_p0, 2), (b_p0, 2), (a_p1, 2),
               (b_p1, 2), (a_u, 4), (b_u, 4)]

    phi30 = np.zeros(30)
    amp30 = np.zeros(30)
    for j, (kappa, b) in enumerate(D_order):
        Rt, Rc = R[A_INFO[b][0]], R[A_INFO[b][1]]
        cs, cc = kappa[0] * Rt[0, 0], kappa[0] * Rt[0, 2]
        amp30[2 * j] = np.hypot(cs, cc)
        phi30[2 * j] = np.arctan2(cc, cs)
        cs = kappa[1] * Rt[1, 0] + kappa[2] * Rt[2, 0]
        cc = kappa[1] * Rt[1, 2] + kappa[2] * Rt[2, 2]
        amp30[2 * j + 1] = np.hypot(cs, cc) * np.hypot(Rc[2, 0], Rc[2, 2])
        phi30[2 * j + 1] = np.arctan2(cc, cs)
    for g, b in enumerate([0, 3, 5, 1, 2, 4]):
        Rc = R[A_INFO[b][1]]
        phi30[24 + g] = np.arctan2(Rc[2, 2], Rc[2, 0])
        amp30[24 + g] = 1.0
    return phi30.astype(np.float32), amp30.astype(np.float32)


# ---------------------------------------------------------------------------
# Walrus workaround: this build rejects >1 sync-wait per instruction
# ---------------------------------------------------------------------------


def _split_multi_waits(nc):
    for f in nc.m.functions:
        for bb in f.blocks:
            out = []
            for inst in bb.instructions:
                si = inst.sync_info
                if si is not None and si.on_wait and len(si.on_wait) > 1:
                    waits = list(si.on_wait)
                    for i, w in enumerate(waits[:-1]):
                        out.append(mybir.InstNoOp(
                            name=f"{inst.name}_wsplit{i}",
                            engine=inst.engine,
                            ins=[], outs=[],
                            sync_info=bass_rust.SyncInfo(
                                on_wait=[w], on_update=[]),
                        ))
                    inst.sync_info = bass_rust.SyncInfo(
                        on_wait=[waits[-1]], on_update=list(si.on_update))
                out.append(inst)
            bb.instructions = out


# ---------------------------------------------------------------------------
# Device kernel
# ---------------------------------------------------------------------------


def _build_nc():
    nc = bass.Bass("TRN2", target_bir_lowering=False, num_devices=N_CORES)

    roa_d = {r: nc.declare_dram_parameter(f"roa_{r}", [P, EPAD], F8,
                                          isOutput=False) for r in "ab"}
    rit_d = {r: nc.declare_dram_parameter(f"rit_{r}", [P, EPAD], F8,
                                          isOutput=False) for r in "ab"}
    xs_d = nc.declare_dram_parameter("xs", [P, NCH * 8], BF16, isOutput=False)
    sm_d = nc.declare_dram_parameter("smalls", [P, SM_W], F32, isOutput=False)
    out_d = nc.declare_dram_parameter("out", [P, 1], F32, isOutput=True)

    MUL = mybir.AluOpType.mult
    ADD = mybir.AluOpType.add

    with tile.TileContext(nc) as tc:
        import contextlib
        scope = (tc.spectator_scope if _SCOPES else
                 (lambda name: contextlib.nullcontext()))
        with (
            tc.tile_pool(name="sb", bufs=1) as sb,
            tc.tile_pool(name="s1p", bufs=4, space="PSUM") as s1p,
            tc.tile_pool(name="accp", bufs=2, space="PSUM") as accp,
        ):
            sc = scope("ld")
            sc.__enter__()
            sm_sb = sb.tile([P, SM_W], F32, name="sm_sb")
            nc.sync.dma_start(sm_sb[:], sm_d[:])
            xs_sb = sb.tile([P, NCH * 8], BF16, name="xs_sb")
            nc.sync.dma_start(xs_sb[:], xs_d[:])
            roa_sb, rit_sb = {}, {}
            for r in "ab":
                roa_sb[r] = sb.tile([P, EPAD], F8, name=f"roa_{r}")
                nc.sync.dma_start(roa_sb[r][:], roa_d[r][:])
                rit_sb[r] = sb.tile([P, EPAD], F8, name=f"rit_{r}")
                nc.gpsimd.dma_start(rit_sb[r][:], rit_d[r][:])

            # preload the ACT Sin table while DMAs stream; warm the PE
            warm = sb.tile([P, 8], F32, name="warm")
            nc.vector.memset(warm[:], 0.0)
            nc.scalar.activation(warm[:, 0:1], warm[:, 0:1],
                                 mybir.ActivationFunctionType.Sin)
            warm16 = sb.tile([P, 8], BF16, name="warm16")
            nc.vector.memset(warm16[:], 0.0)
            for i in range(3):
                wp = s1p.tile([P, 8], F32, name=f"warm_ps{i}", tag="s1")
                nc.tensor.matmul(wp[:], roa_sb["a"][:, 0:P], warm16[:],
                                 start=True, stop=True)
            sc.__exit__(None, None, None)

            # ---- stage 1 + 2 per relation --------------------------------
            sc = scope("s1")
            sc.__enter__()
            ENG = [nc.vector, nc.gpsimd]
            bhl = {}
            for ri, r in enumerate("ab"):
                beo = sb.tile([P, NB * D], F32, name=f"beo_{r}")
                for m in range(NB):
                    ps = s1p.tile([P, 8], F32, name=f"s1ps_{r}{m}", tag="s1")
                    nc.tensor.matmul(
                        ps[:], roa_sb[r][:, m * P:(m + 1) * P],
                        xs_sb[:, (m // 2) * 8:(m // 2) * 8 + 8],
                        start=True, stop=True)
                    # only one PSUM operand per op, and GPSIMD can't read
                    # PSUM: hi via scalar copy to SBUF, vector adds lo
                    tmp = sb.tile([P, 4], F32, name=f"ev_{r}{m}", tag="ev",
                                  bufs=4)
                    nc.scalar.copy(tmp[:], ps[:, 0:4])
                    nc.vector.tensor_tensor(
                        beo[:, m * D:(m + 1) * D], tmp[:], ps[:, 4:8], ADD)
                eoff = SM_EA if ri == 0 else SM_EB
                nc.gpsimd.tensor_tensor(
                    beo[:], beo[:], sm_sb[:, eoff:eoff + NB * D], MUL)
                # split into bf16 hi/lo pairs [hi(4) | lo(4)] per j-block
                hl = sb.tile([P, NB * 8], BF16, name=f"bhl_{r}")
                hl3 = hl.rearrange("p (m c) -> p m c", c=8)
                hi_view, lo_view = hl3[:, :, 0:4], hl3[:, :, 4:8]
                beo3 = beo.rearrange("p (m d) -> p m d", d=D)
                brs = sb.tile([P, NB * D], F32, name=f"brs_{r}")
                brs3 = brs.rearrange("p (m d) -> p m d", d=D)
                nc.gpsimd.tensor_copy(hi_view, beo3)
                nc.vector.scalar_tensor_tensor(
                    brs3, hi_view, -1.0, beo3, MUL, ADD)
                nc.gpsimd.tensor_copy(lo_view, brs3)
                bhl[r] = hl
            sc.__exit__(None, None, None)

            # ---- stage 3: scatter to own nodes ---------------------------
            sc = scope("s3")
            sc.__enter__()
            mm = sb.tile([P, 8], F32, name="mm")
            for ri, r in enumerate("ab"):
                acc = accp.tile([P, 8], F32, name=f"acc_{r}", tag=f"acc{r}")
                for m in range(NB):
                    nc.tensor.matmul(
                        acc[:], rit_sb[r][:, m * P:(m + 1) * P],
                        bhl[r][:, m * 8:(m + 1) * 8],
                        start=(m == 0), stop=(m == NB - 1))
                tmp = sb.tile([P, 4], F32, name=f"s3ev_{r}", tag="ev",
                              bufs=4)
                nc.scalar.copy(tmp[:], acc[:, 0:4])
                nc.vector.tensor_tensor(
                    mm[:, ri * 4:(ri + 1) * 4], tmp[:], acc[:, 4:8], ADD)
            sc.__exit__(None, None, None)

            # ---- circuit ------------------------------------------------
            sc = scope("ci")
            sc.__enter__()

            def phi(s, w):
                return sm_sb[:, SM_PHI + s:SM_PHI + s + w]

            # m30 build: per group, angle col broadcast + phi
            m30 = sb.tile([P, 30], F32, name="m30")
            # AP-scalar (Ptr) ops only run on DVE; ACT takes the bias form
            BENG = [nc.vector, nc.scalar]
            # target angle source per group: wires [1, 6, 10, 2, 5, 9]
            # -> [mi1, mo2, X2, mi2, mo1, X1]
            tsrc = [mm[:, 1:2], mm[:, 6:7], sm_sb[:, SM_X + 2:SM_X + 3],
                    mm[:, 2:3], mm[:, 5:6], sm_sb[:, SM_X + 1:SM_X + 2]]
            for g, (s, w, _b) in enumerate(_GROUPS):
                eng = BENG[g % 2]
                if eng is nc.scalar:
                    eng.add(m30[:, s:s + w], phi(s, w), tsrc[g])
                else:
                    eng.tensor_scalar(m30[:, s:s + w], phi(s, w), tsrc[g],
                                      None, ADD)
            # ctrl cols 24:30 <- wires [0, 7, 11, 3, 4, 8]
            csrc = [mm[:, 0:1], mm[:, 7:8], sm_sb[:, SM_X + 3:SM_X + 4],
                    mm[:, 3:4], mm[:, 4:5], sm_sb[:, SM_X:SM_X + 1]]
            for g in range(6):
                s = 24 + g
                eng = BENG[g % 2]
                if eng is nc.scalar:
                    eng.add(m30[:, s:s + 1], phi(s, 1), csrc[g])
                else:
                    eng.tensor_scalar(m30[:, s:s + 1], phi(s, 1), csrc[g],
                                      None, ADD)

            # wrap into [-pi, pi] (|m+phi| < 5*pi so two wraps suffice)
            if _FUSED_DVE:
                nc.vector.add_range_wrap(m30[:], m30[:], 0.0, PI, 2 * PI)
                nc.vector.add_range_wrap(m30[:], m30[:], 0.0, PI, 2 * PI)
            else:
                # rne(m/2pi) via f32->i32 cast (RNE on the DVE), then
                # m - 2pi*k, clamped
                t_f = sb.tile([P, 30], F32, name="t_f")
                t_i = sb.tile([P, 30], mybir.dt.int32, name="t_i")
                t_r = sb.tile([P, 30], F32, name="t_r")
                nc.vector.tensor_scalar(
                    t_f[:], m30[:], float(1.0 / (2 * PI)), None, MUL)
                nc.vector.tensor_copy(t_i[:], t_f[:])
                nc.vector.tensor_copy(t_r[:], t_i[:])
                nc.vector.scalar_tensor_tensor(
                    m30[:], t_r[:], -2 * PI, m30[:], MUL, ADD)
                nc.vector.tensor_scalar(
                    m30[:], m30[:], PI, -PI,
                    mybir.AluOpType.min, mybir.AluOpType.max)

            s30 = sb.tile([P, 30], F32, name="s30")
            nc.scalar.activation(s30[:], m30[:],
                                 mybir.ActivationFunctionType.Sin)
            nc.vector.tensor_tensor(
                s30[:], s30[:], sm_sb[:, SM_AMP:SM_AMP + 30], MUL)

            # D12 = q + az * r per group
            d12 = sb.tile([P, 12], F32, name="d12")
            dpos = 0
            for g, (s, w, _b) in enumerate(_GROUPS):
                nd = w // 2
                if _FUSED_DVE:
                    nc.vector.affine_then_add(
                        d12[:, dpos:dpos + nd],
                        s30[:, s + 1:s + w:2], s30[:, s:s + w:2],
                        s30[:, 24 + g:24 + g + 1], 0.0)
                else:
                    nc.vector.tensor_scalar(
                        d12[:, dpos:dpos + nd], s30[:, s + 1:s + w:2],
                        s30[:, 24 + g:24 + g + 1], None, MUL)
                    ENG[g % 2].tensor_tensor(
                        d12[:, dpos:dpos + nd], d12[:, dpos:dpos + nd],
                        s30[:, s:s + w:2], ADD)
                dpos += nd

            # F = [s9, p0, p1, u]; z9 = (p0 + s9*p1)*u
            f4 = sb.tile([P, 8], F32, name="f4")
            nc.vector.tensor_tensor(f4[:, 4:8], d12[:, 5:12:2], d12[:, 0:4],
                                    MUL)
            nc.vector.tensor_tensor(f4[:, 0:4], d12[:, 4:12:2], f4[:, 4:8],
                                    ADD)
            t2 = sb.tile([P, 2], F32, name="t2")
            nc.vector.tensor_tensor(t2[:, 0:1], f4[:, 0:1], f4[:, 2:3], MUL)
            nc.vector.tensor_tensor(t2[:, 1:2], f4[:, 1:2], t2[:, 0:1], ADD)
            res = sb.tile([P, 1], F32, name="res")
            nc.vector.tensor_tensor(res[:], t2[:, 1:2], f4[:, 3:4], MUL)
            nc.vector.tensor_scalar(res[:], res[:], -PI, PI, MUL, ADD)
            nc.sync.dma_start(out_d[:], res[:])
            sc.__exit__(None, None, None)

    return nc


_NC_CACHE = {}
_RUN_KWARGS = {}      # test harness can set e.g. {"trace": True}
_LAST_RESULTS = []    # BassKernelResults of the most recent run


def _get_nc():
    if "nc" not in _NC_CACHE:
        nc = _build_nc()
        _split_multi_waits(nc)
        _NC_CACHE["nc"] = nc
    return _NC_CACHE["nc"]


def _shard_rel(idx_t, idx_s, e):
    """Per-core (roa, rita, e64) for one relation.

    Core k owns edges with idx_t in its node slice, sorted by source
    chunk; runs padded to RPAD. roa[src%128, j] = 1 (stage-1 stationary,
    partition = node-in-chunk), rita[j%128, (j//128)*128 + tgt%128] = 1
    (stage-3 stationary, partition = j-in-block, host p-major layout).
    """
    f8 = ml_dtypes.float8_e4m3fn
    outs = []
    for k in range(N_CORES):
        ed = np.where(idx_t // P == k)[0]
        sc = idx_s[ed] // P
        order = np.argsort(sc, kind="stable")
        ed, sc = ed[order], sc[order]
        counts = np.bincount(sc, minlength=NCH)
        if counts.max() > RPAD:
            raise ValueError(f"source-chunk run {counts.max()} > RPAD")
        starts = np.searchsorted(sc, np.arange(NCH))
        j = np.arange(len(ed)) - starts[sc] + sc * RPAD
        roa = np.zeros((P, EPAD), np.float32)
        roa[idx_s[ed] % P, j] = 1.0
        rita = np.zeros((P, EPAD), np.float32)
        rita[j % P, (j // P) * P + idx_t[ed] % P] = 1.0
        e16 = np.zeros((P, NB), np.float32)
        e16[j % P, j // P] = e[ed]
        outs.append((roa.astype(f8), rita.astype(f8),
                     np.repeat(e16, D, axis=1)))
    return outs


def kernel(X, e, Ri, Ro, theta):
    X = np.ascontiguousarray(np.asarray(X, np.float32))
    e = np.ascontiguousarray(np.asarray(e, np.float32))
    Ri = np.asarray(Ri, np.float32)
    Ro = np.asarray(Ro, np.float32)
    theta = np.asarray(theta, np.float32)

    bf = ml_dtypes.bfloat16
    idx_i = np.argmax(Ri, axis=0)
    idx_o = np.argmax(Ro, axis=0)

    # rel A feeds mi (targets idx_i, sources idx_o); rel B feeds mo
    sh_a = _shard_rel(idx_i, idx_o, e)
    sh_b = _shard_rel(idx_o, idx_i, e)

    xh = X.astype(bf).astype(np.float32)
    xl = X - xh
    xs = np.zeros((P, NCH, 8), np.float32)
    xs[:, :, 0:4] = xh.reshape(NCH, P, D).transpose(1, 0, 2)
    xs[:, :, 4:8] = xl.reshape(NCH, P, D).transpose(1, 0, 2)
    xs = np.ascontiguousarray(xs.reshape(P, NCH * 8).astype(bf))

    phi30, amp30 = _pack_tables(theta)

    in_maps = []
    for k in range(N_CORES):
        sm = np.zeros((P, SM_W), np.float32)
        sm[:, SM_X:SM_X + 4] = X[k * P:(k + 1) * P]
        sm[:, SM_EA:SM_EA + NB * D] = sh_a[k][2]
        sm[:, SM_EB:SM_EB + NB * D] = sh_b[k][2]
        sm[:, SM_PHI:SM_PHI + 30] = phi30[None, :]
        sm[:, SM_AMP:SM_AMP + 30] = amp30[None, :]
        in_maps.append({
            "roa_a": np.ascontiguousarray(sh_a[k][0]),
            "rit_a": np.ascontiguousarray(sh_a[k][1]),
            "roa_b": np.ascontiguousarray(sh_b[k][0]),
            "rit_b": np.ascontiguousarray(sh_b[k][1]),
            "xs": xs,
            "smalls": np.ascontiguousarray(sm),
        })

    nc = _get_nc()
    res = run_bass_kernel_spmd(nc, in_maps, core_ids=list(range(N_CORES)),
                               **_RUN_KWARGS)
    _LAST_RESULTS.clear()
    _LAST_RESULTS.append(res)
    return np.concatenate(
        [res.results[k]["out"].reshape(-1) for k in range(N_CORES)]
    ).astype(np.float32)


# revision 21
# speedup vs baseline: 3.2842x; 1.1048x over previous
"""Trainium2 Bass kernel for nn_NodeNet: GNN message passing + 12-qubit TTN.

Collective-free sharding: the host owns the edge partition, so core k
receives exactly the edges whose TARGET node lands in its 128-node
slice - once for mi (targets = idx_i) and once for mo (targets =
idx_o). Both contractions are then fully local:

  stage 1 (gather):  beo[j] = e_j * X[src_j]   as a matmul against the
      one-hot source matrix, chunked 128x128 so each chunk's stationary
      is the fp8 0/1 block (exact) and the moving operand is X hi/lo
      bf16 [128, 8]. Edges are sorted by source chunk; per-chunk runs
      are padded to RPAD=256 (max real run ~170).
  stage 2: evict hi+lo, scale by e (f32), re-split to bf16 hi/lo.
  stage 3 (scatter): mi[n] = sum_j beo[j] * RiT[j, n] with stationary =
      the one-hot target block [128, 128] fp8 and moving = beo hi/lo
      [128, 8]; PSUM accumulates over the 16 j-blocks and the output is
      node-partitioned [128, 8] directly - no transposes, no collective.

The circuit contracts to per-node Bloch chains (CNOT target keeps
(x, az*y, az*z); measurement is <Z_9>). Every linear term collapses to
A*sin(m + phi) with host-precomputed amplitude/phase, so the whole
chain is ~30 wide vector ops: build 30 angle columns, wrap to [-pi,pi],
one Sin activation, amplitude multiply, 6 fused (q + az*r) ops, and a
5-op tail.
"""

import ml_dtypes
import numpy as np

import bass_rust
import concourse.bass as bass
import concourse.mybir as mybir
import concourse.tile as tile
from concourse.bass_utils import run_bass_kernel_spmd

F32 = mybir.dt.float32
BF16 = mybir.dt.bfloat16
F8 = mybir.dt.float8e4
N_CORES = 8
N, E, D = 1024, 8192, 4
P = 128                  # partitions / nodes per core
NCH = N // P             # 8 node chunks
RPAD = 256               # padded edges per source chunk (max real ~170)
EPAD = NCH * RPAD        # 2048 padded edges per core per relation
NB = EPAD // P           # 16 j-blocks
PI = float(np.pi)

_SCOPES = False          # test harness can flip on for phase attribution
_FUSED_DVE = False       # use custom-DVE fused ops (add_range_wrap etc.)

_BLOCKS = [(0, 1, (0, 1)), (2, 3, (3, 2)), (4, 5, (4, 5)), (6, 7, (7, 6)),
           (8, 9, (8, 9)), (10, 11, (11, 10)), (1, 2, (1, 2)), (5, 6, (6, 5)),
           (9, 10, (10, 9)), (2, 5, (2, 5)), (5, 9, (5, 9))]

# A-layer blocks 0..5: (target rot idx, ctrl rot idx, target wire, ctrl wire)
A_INFO = []
for _b, (_w1, _w2, (_c, _t)) in enumerate(_BLOCKS[:6]):
    A_INFO.append((2 * _b if _t == _w1 else 2 * _b + 1,
                   2 * _b if _c == _w1 else 2 * _b + 1, _t, _c))

# sin30 layout: 6 groups of [q,r]-pairs over A-blocks [0,3,5,1,2,4]
# (b3 and b2 carry 2 and 4 D's), then 6 ctrl cols.
# D12 = [az6, az7, az7, az8, s9a, s9b, p0a, p0b, p1a, p1b, ua, ub]
_GROUPS = [(0, 2, 0), (2, 4, 3), (6, 2, 5), (8, 4, 1), (12, 8, 2),
           (20, 4, 4)]          # (m30 start, width, A-block)

# smalls column layout
SM_X = 0                 # own-node X angle cols (4)
SM_EA = 4                # e for rel A, per j-block col: [128, 16]
SM_EB = 20
SM_PHI = 36              # phi30
SM_AMP = 66              # amp30
SM_W = 96

# ---------------------------------------------------------------------------
# Host-side circuit-constant preparation
# ---------------------------------------------------------------------------

_PAULI = np.array([
    [[0, 1], [1, 0]],
    [[0, -1j], [1j, 0]],
    [[1, 0], [0, -1]],
], dtype=np.complex128)


def _rot_so3(p):
    """SO(3) Bloch rotation of Rot(phi, theta, omega) = RZ(om) RY(th) RZ(phi)."""
    phi, th, om = float(p[0]), float(p[1]), float(p[2])
    c, s = np.cos(th / 2), np.sin(th / 2)
    U = np.array([
        [np.exp(-0.5j * (phi + om)) * c, -np.exp(0.5j * (phi - om)) * s],
        [np.exp(-0.5j * (phi - om)) * s, np.exp(0.5j * (phi + om)) * c],
    ])
    R = np.empty((3, 3))
    for i in range(3):
        for j in range(3):
            R[i, j] = 0.5 * np.real(
                np.trace(_PAULI[i] @ U @ _PAULI[j] @ U.conj().T))
    return R


def _pack_tables(theta):
    """phi30/amp30 for the amplitude-phase sin tile (see module docstring)."""
    th = np.asarray(theta, np.float64)
    R = [_rot_so3(th[3 * k:3 * k + 3]) for k in range(23)]

    def split_ab(row2, Rt):
        return row2[0] * Rt[0, :], row2[1] * Rt[1, :] + row2[2] * Rt[2, :]

    a_s9, b_s9 = split_ab(R[18][2], R[13])
    v0 = R[20][2, 0] * R[19][0, :]
    v1 = R[20][2, 1] * R[19][1, :] + R[20][2, 2] * R[19][2, :]
    a_p0, b_p0 = split_ab(v0, R[14])
    a_p1, b_p1 = split_ab(v1, R[14])
    a_u, b_u = split_ab(R[21][2], R[16])

    D_order = [(R[12][2], 0), (R[15][2], 3), (R[15][2], 3), (R[17][2], 5),
               (a_s9, 1), (b_s9, 1), (a_p0, 2), (b_p0, 2), (a_p1, 2),
               (b_p1, 2), (a_u, 4), (b_u, 4)]

    phi30 = np.zeros(30)
    amp30 = np.zeros(30)
    for j, (kappa, b) in enumerate(D_order):
        Rt, Rc = R[A_INFO[b][0]], R[A_INFO[b][1]]
        cs, cc = kappa[0] * Rt[0, 0], kappa[0] * Rt[0, 2]
        amp30[2 * j] = np.hypot(cs, cc)
        phi30[2 * j] = np.arctan2(cc, cs)
        cs = kappa[1] * Rt[1, 0] + kappa[2] * Rt[2, 0]
        cc = kappa[1] * Rt[1, 2] + kappa[2] * Rt[2, 2]
        amp30[2 * j + 1] = np.hypot(cs, cc) * np.hypot(Rc[2, 0], Rc[2, 2])
        phi30[2 * j + 1] = np.arctan2(cc, cs)
    for g, b in enumerate([0, 3, 5, 1, 2, 4]):
        Rc = R[A_INFO[b][1]]
        phi30[24 + g] = np.arctan2(Rc[2, 2], Rc[2, 0])
        amp30[24 + g] = 1.0
    return phi30.astype(np.float32), amp30.astype(np.float32)


# ---------------------------------------------------------------------------
# Walrus workaround: this build rejects >1 sync-wait per instruction
# ---------------------------------------------------------------------------


def _split_multi_waits(nc):
    for f in nc.m.functions:
        for bb in f.blocks:
            out = []
            for inst in bb.instructions:
                si = inst.sync_info
                if si is not None and si.on_wait and len(si.on_wait) > 1:
                    waits = list(si.on_wait)
                    for i, w in enumerate(waits[:-1]):
                        out.append(mybir.InstNoOp(
                            name=f"{inst.name}_wsplit{i}",
                            engine=inst.engine,
                            ins=[], outs=[],
                            sync_info=bass_rust.SyncInfo(
                                on_wait=[w], on_update=[]),
                        ))
                    inst.sync_info = bass_rust.SyncInfo(
                        on_wait=[waits[-1]], on_update=list(si.on_update))
                out.append(inst)
            bb.instructions = out


# ---------------------------------------------------------------------------
# Device kernel
# ---------------------------------------------------------------------------


def _build_nc():
    nc = bass.Bass("TRN2", target_bir_lowering=False, num_devices=N_CORES)

    roa_d = {r: nc.declare_dram_parameter(f"roa_{r}", [P, EPAD], F8,
                                          isOutput=False) for r in "ab"}
    rit_d = {r: nc.declare_dram_parameter(f"rit_{r}", [P, EPAD], F8,
                                          isOutput=False) for r in "ab"}
    xs_d = nc.declare_dram_parameter("xs", [P, NCH * 8], BF16, isOutput=False)
    sm_d = nc.declare_dram_parameter("smalls", [P, SM_W], F32, isOutput=False)
    out_d = nc.declare_dram_parameter("out", [P, 1], F32, isOutput=True)

    MUL = mybir.AluOpType.mult
    ADD = mybir.AluOpType.add

    with tile.TileContext(nc) as tc:
        import contextlib
        scope = (tc.spectator_scope if _SCOPES else
                 (lambda name: contextlib.nullcontext()))
        with (
            tc.tile_pool(name="sb", bufs=1) as sb,
            tc.tile_pool(name="s1p", bufs=4, space="PSUM") as s1p,
            tc.tile_pool(name="accp", bufs=2, space="PSUM") as accp,
        ):
            sc = scope("ld")
            sc.__enter__()
            roa_sb, rit_sb = {}, {}
            for r in "ab":
                roa_sb[r] = sb.tile([P, EPAD], F8, name=f"roa_{r}")
                rit_sb[r] = sb.tile([P, EPAD], F8, name=f"rit_{r}")
            nc.sync.dma_start(roa_sb["a"][:], roa_d["a"][:])
            xs_sb = sb.tile([P, NCH * 8], BF16, name="xs_sb")
            nc.sync.dma_start(xs_sb[:], xs_d[:])
            sm_sb = sb.tile([P, SM_W], F32, name="sm_sb")
            nc.sync.dma_start(sm_sb[:], sm_d[:])
            nc.sync.dma_start(roa_sb["b"][:], roa_d["b"][:])
            for r in "ab":
                nc.gpsimd.dma_start(rit_sb[r][:], rit_d[r][:])

            # preload the ACT Sin table while DMAs stream; warm the PE
            warm = sb.tile([P, 8], F32, name="warm")
            nc.vector.memset(warm[:], 0.0)
            nc.scalar.activation(warm[:, 0:1], warm[:, 0:1],
                                 mybir.ActivationFunctionType.Sin)
            warm16 = sb.tile([P, P], BF16, name="warm16")
            nc.vector.memset(warm16[:], 0.0)
            for i in range(3):
                wp = s1p.tile([P, 8], F32, name=f"warm_ps{i}", tag="s1")
                nc.tensor.matmul(wp[:], warm16[:], warm16[:, 0:8],
                                 start=True, stop=True)
            sc.__exit__(None, None, None)

            # ---- stage 1 + 2 per relation --------------------------------
            sc = scope("s1")
            sc.__enter__()
            ENG = [nc.vector, nc.gpsimd]
            bhl = {}
            for ri, r in enumerate("ab"):
                beo = sb.tile([P, NB * D], F32, name=f"beo_{r}")
                eoff = SM_EA if ri == 0 else SM_EB
                for m in range(NB):
                    ps = s1p.tile([P, D], F32, name=f"s1ps_{r}{m}", tag="s1")
                    c8 = (m // 2) * 8
                    nc.tensor.matmul(
                        ps[:], roa_sb[r][:, m * P:(m + 1) * P],
                        xs_sb[:, c8:c8 + 4], start=True, stop=False)
                    nc.tensor.matmul(
                        ps[:], roa_sb[r][:, m * P:(m + 1) * P],
                        xs_sb[:, c8 + 4:c8 + 8], start=False, stop=True)
                    nc.vector.tensor_scalar(
                        beo[:, m * D:(m + 1) * D], ps[:],
                        sm_sb[:, eoff + m:eoff + m + 1], None, MUL)
                # split into bf16 hi/lo pairs [hi(4) | lo(4)] per j-block
                hl = sb.tile([P, NB * 8], BF16, name=f"bhl_{r}")
                hl3 = hl.rearrange("p (m c) -> p m c", c=8)
                hi_view, lo_view = hl3[:, :, 0:4], hl3[:, :, 4:8]
                beo3 = beo.rearrange("p (m d) -> p m d", d=D)
                brs = sb.tile([P, NB * D], F32, name=f"brs_{r}")
                brs3 = brs.rearrange("p (m d) -> p m d", d=D)
                nc.gpsimd.tensor_copy(hi_view, beo3)
                nc.vector.scalar_tensor_tensor(
                    brs3, hi_view, -1.0, beo3, MUL, ADD)
                nc.gpsimd.tensor_copy(lo_view, brs3)
                bhl[r] = hl
            sc.__exit__(None, None, None)

            # ---- stage 3: scatter to own nodes ---------------------------
            sc = scope("s3")
            sc.__enter__()
            mm = sb.tile([P, 8], F32, name="mm")
            for ri, r in enumerate("ab"):
                acc = accp.tile([P, 8], F32, name=f"acc_{r}", tag=f"acc{r}")
                for m in range(NB):
                    nc.tensor.matmul(
                        acc[:], rit_sb[r][:, m * P:(m + 1) * P],
                        bhl[r][:, m * 8:(m + 1) * 8],
                        start=(m == 0), stop=(m == NB - 1))
                tmp = sb.tile([P, 4], F32, name=f"s3ev_{r}", tag="ev",
                              bufs=4)
                nc.vector.tensor_copy(tmp[:], acc[:, 0:4])
                nc.vector.tensor_tensor(
                    mm[:, ri * 4:(ri + 1) * 4], tmp[:], acc[:, 4:8], ADD)
            sc.__exit__(None, None, None)

            # ---- circuit ------------------------------------------------
            sc = scope("ci")
            sc.__enter__()

            def phi(s, w):
                return sm_sb[:, SM_PHI + s:SM_PHI + s + w]

            # m30 build: per group, angle col broadcast + phi
            m30 = sb.tile([P, 30], F32, name="m30")
            # AP-scalar (Ptr) ops only run on DVE; ACT takes the bias form
            BENG = [nc.vector, nc.scalar]
            # target angle source per group: wires [1, 6, 10, 2, 5, 9]
            # -> [mi1, mo2, X2, mi2, mo1, X1]
            tsrc = [mm[:, 1:2], mm[:, 6:7], sm_sb[:, SM_X + 2:SM_X + 3],
                    mm[:, 2:3], mm[:, 5:6], sm_sb[:, SM_X + 1:SM_X + 2]]
            for g, (s, w, _b) in enumerate(_GROUPS):
                eng = BENG[g % 2]
                if eng is nc.scalar:
                    eng.add(m30[:, s:s + w], phi(s, w), tsrc[g])
                else:
                    eng.tensor_scalar(m30[:, s:s + w], phi(s, w), tsrc[g],
                                      None, ADD)
            # ctrl cols 24:30 <- wires [0, 7, 11, 3, 4, 8]
            csrc = [mm[:, 0:1], mm[:, 7:8], sm_sb[:, SM_X + 3:SM_X + 4],
                    mm[:, 3:4], mm[:, 4:5], sm_sb[:, SM_X:SM_X + 1]]
            for g in range(6):
                s = 24 + g
                eng = BENG[g % 2]
                if eng is nc.scalar:
                    eng.add(m30[:, s:s + 1], phi(s, 1), csrc[g])
                else:
                    eng.tensor_scalar(m30[:, s:s + 1], phi(s, 1), csrc[g],
                                      None, ADD)

            # wrap into [-pi, pi] (|m+phi| < 5*pi so two wraps suffice)
            if _FUSED_DVE:
                nc.vector.add_range_wrap(m30[:], m30[:], 0.0, PI, 2 * PI)
                nc.vector.add_range_wrap(m30[:], m30[:], 0.0, PI, 2 * PI)
            else:
                # rne(m/2pi) via f32->i32 cast (RNE on the DVE), then
                # m - 2pi*k, clamped
                t_f = sb.tile([P, 30], F32, name="t_f")
                t_i = sb.tile([P, 30], mybir.dt.int32, name="t_i")
                t_r = sb.tile([P, 30], F32, name="t_r")
                nc.vector.tensor_scalar(
                    t_f[:], m30[:], float(1.0 / (2 * PI)), None, MUL)
                nc.vector.tensor_copy(t_i[:], t_f[:])
                nc.vector.tensor_copy(t_r[:], t_i[:])
                nc.vector.scalar_tensor_tensor(
                    m30[:], t_r[:], -2 * PI, m30[:], MUL, ADD)
                nc.vector.tensor_scalar(
                    m30[:], m30[:], PI, -PI,
                    mybir.AluOpType.min, mybir.AluOpType.max)

            s30 = sb.tile([P, 30], F32, name="s30")
            nc.scalar.activation(s30[:], m30[:],
                                 mybir.ActivationFunctionType.Sin)
            nc.vector.tensor_tensor(
                s30[:], s30[:], sm_sb[:, SM_AMP:SM_AMP + 30], MUL)

            # D12 = q + az * r per group
            d12 = sb.tile([P, 12], F32, name="d12")
            dpos = 0
            for g, (s, w, _b) in enumerate(_GROUPS):
                nd = w // 2
                if _FUSED_DVE:
                    nc.vector.affine_then_add(
                        d12[:, dpos:dpos + nd],
                        s30[:, s + 1:s + w:2], s30[:, s:s + w:2],
                        s30[:, 24 + g:24 + g + 1], 0.0)
                else:
                    nc.vector.tensor_scalar(
                        d12[:, dpos:dpos + nd], s30[:, s + 1:s + w:2],
                        s30[:, 24 + g:24 + g + 1], None, MUL)
                    ENG[g % 2].tensor_tensor(
                        d12[:, dpos:dpos + nd], d12[:, dpos:dpos + nd],
                        s30[:, s:s + w:2], ADD)
                dpos += nd

            # F = [s9, p0, p1, u]; z9 = (p0 + s9*p1)*u
            f4 = sb.tile([P, 8], F32, name="f4")
            nc.vector.tensor_tensor(f4[:, 4:8], d12[:, 5:12:2], d12[:, 0:4],
                                    MUL)
            nc.vector.tensor_tensor(f4[:, 0:4], d12[:, 4:12:2], f4[:, 4:8],
                                    ADD)
            t2 = sb.tile([P, 2], F32, name="t2")
            nc.vector.tensor_tensor(t2[:, 0:1], f4[:, 0:1], f4[:, 2:3], MUL)
            nc.vector.tensor_tensor(t2[:, 1:2], f4[:, 1:2], t2[:, 0:1], ADD)
            res = sb.tile([P, 1], F32, name="res")
            nc.vector.tensor_tensor(res[:], t2[:, 1:2], f4[:, 3:4], MUL)
            nc.vector.tensor_scalar(res[:], res[:], -PI, PI, MUL, ADD)
            nc.sync.dma_start(out_d[:], res[:])
            sc.__exit__(None, None, None)

    return nc


_NC_CACHE = {}
_RUN_KWARGS = {}      # test harness can set e.g. {"trace": True}
_LAST_RESULTS = []    # BassKernelResults of the most recent run


def _get_nc():
    if "nc" not in _NC_CACHE:
        nc = _build_nc()
        _split_multi_waits(nc)
        _NC_CACHE["nc"] = nc
    return _NC_CACHE["nc"]


def _shard_rel(idx_t, idx_s, e):
    """Per-core (roa, rita, e64) for one relation.

    Core k owns edges with idx_t in its node slice, sorted by source
    chunk; runs padded to RPAD. roa[src%128, j] = 1 (stage-1 stationary,
    partition = node-in-chunk), rita[j%128, (j//128)*128 + tgt%128] = 1
    (stage-3 stationary, partition = j-in-block, host p-major layout).
    """
    f8 = ml_dtypes.float8_e4m3fn
    outs = []
    for k in range(N_CORES):
        ed = np.where(idx_t // P == k)[0]
        sc = idx_s[ed] // P
        order = np.argsort(sc, kind="stable")
        ed, sc = ed[order], sc[order]
        counts = np.bincount(sc, minlength=NCH)
        if counts.max() > RPAD:
            raise ValueError(f"source-chunk run {counts.max()} > RPAD")
        starts = np.searchsorted(sc, np.arange(NCH))
        j = np.arange(len(ed)) - starts[sc] + sc * RPAD
        roa = np.zeros((P, EPAD), np.float32)
        roa[idx_s[ed] % P, j] = 1.0
        rita = np.zeros((P, EPAD), np.float32)
        rita[j % P, (j // P) * P + idx_t[ed] % P] = 1.0
        e16 = np.zeros((P, NB), np.float32)
        e16[j % P, j // P] = e[ed]
        outs.append((roa.astype(f8), rita.astype(f8), e16))
    return outs


def kernel(X, e, Ri, Ro, theta):
    X = np.ascontiguousarray(np.asarray(X, np.float32))
    e = np.ascontiguousarray(np.asarray(e, np.float32))
    Ri = np.asarray(Ri, np.float32)
    Ro = np.asarray(Ro, np.float32)
    theta = np.asarray(theta, np.float32)

    bf = ml_dtypes.bfloat16
    idx_i = np.argmax(Ri, axis=0)
    idx_o = np.argmax(Ro, axis=0)

    # rel A feeds mi (targets idx_i, sources idx_o); rel B feeds mo
    sh_a = _shard_rel(idx_i, idx_o, e)
    sh_b = _shard_rel(idx_o, idx_i, e)

    xh = X.astype(bf).astype(np.float32)
    xl = X - xh
    xs = np.zeros((P, NCH, 8), np.float32)
    xs[:, :, 0:4] = xh.reshape(NCH, P, D).transpose(1, 0, 2)
    xs[:, :, 4:8] = xl.reshape(NCH, P, D).transpose(1, 0, 2)
    xs = np.ascontiguousarray(xs.reshape(P, NCH * 8).astype(bf))

    phi30, amp30 = _pack_tables(theta)

    in_maps = []
    for k in range(N_CORES):
        sm = np.zeros((P, SM_W), np.float32)
        sm[:, SM_X:SM_X + 4] = X[k * P:(k + 1) * P]
        sm[:, SM_EA:SM_EA + NB] = sh_a[k][2]
        sm[:, SM_EB:SM_EB + NB] = sh_b[k][2]
        sm[:, SM_PHI:SM_PHI + 30] = phi30[None, :]
        sm[:, SM_AMP:SM_AMP + 30] = amp30[None, :]
        in_maps.append({
            "roa_a": np.ascontiguousarray(sh_a[k][0]),
            "rit_a": np.ascontiguousarray(sh_a[k][1]),
            "roa_b": np.ascontiguousarray(sh_b[k][0]),
            "rit_b": np.ascontiguousarray(sh_b[k][1]),
            "xs": xs,
            "smalls": np.ascontiguousarray(sm),
        })

    nc = _get_nc()
    res = run_bass_kernel_spmd(nc, in_maps, core_ids=list(range(N_CORES)),
                               **_RUN_KWARGS)
    _LAST_RESULTS.clear()
    _LAST_RESULTS.append(res)
    return np.concatenate(
        [res.results[k]["out"].reshape(-1) for k in range(N_CORES)]
    ).astype(np.float32)


# revision 22
# speedup vs baseline: 3.3079x; 1.0072x over previous
"""Trainium2 Bass kernel for nn_NodeNet: GNN message passing + 12-qubit TTN.

Collective-free sharding: the host owns the edge partition, so core k
receives exactly the edges whose TARGET node lands in its 128-node
slice - once for mi (targets = idx_i) and once for mo (targets =
idx_o). Both contractions are then fully local:

  stage 1 (gather):  beo[j] = e_j * X[src_j]   as a matmul against the
      one-hot source matrix, chunked 128x128 so each chunk's stationary
      is the fp8 0/1 block (exact) and the moving operand is X hi/lo
      bf16 [128, 8]. Edges are sorted by source chunk; per-chunk runs
      are padded to RPAD=256 (max real run ~170).
  stage 2: evict hi+lo, scale by e (f32), re-split to bf16 hi/lo.
  stage 3 (scatter): mi[n] = sum_j beo[j] * RiT[j, n] with stationary =
      the one-hot target block [128, 128] fp8 and moving = beo hi/lo
      [128, 8]; PSUM accumulates over the 16 j-blocks and the output is
      node-partitioned [128, 8] directly - no transposes, no collective.

The circuit contracts to per-node Bloch chains (CNOT target keeps
(x, az*y, az*z); measurement is <Z_9>). Every linear term collapses to
A*sin(m + phi) with host-precomputed amplitude/phase, so the whole
chain is ~30 wide vector ops: build 30 angle columns, wrap to [-pi,pi],
one Sin activation, amplitude multiply, 6 fused (q + az*r) ops, and a
5-op tail.
"""

import ml_dtypes
import numpy as np

import bass_rust
import concourse.bass as bass
import concourse.mybir as mybir
import concourse.tile as tile
from concourse.bass_utils import run_bass_kernel_spmd

F32 = mybir.dt.float32
BF16 = mybir.dt.bfloat16
F8 = mybir.dt.float8e4
N_CORES = 8
N, E, D = 1024, 8192, 4
P = 128                  # partitions / nodes per core
NCH = N // P             # 8 node chunks
RPAD = 256               # padded edges per source chunk (max real ~170)
EPAD = NCH * RPAD        # 2048 padded edges per core per relation
NB = EPAD // P           # 16 j-blocks
PI = float(np.pi)

_SCOPES = False          # test harness can flip on for phase attribution
_FUSED_DVE = False       # use custom-DVE fused ops (add_range_wrap etc.)

_BLOCKS = [(0, 1, (0, 1)), (2, 3, (3, 2)), (4, 5, (4, 5)), (6, 7, (7, 6)),
           (8, 9, (8, 9)), (10, 11, (11, 10)), (1, 2, (1, 2)), (5, 6, (6, 5)),
           (9, 10, (10, 9)), (2, 5, (2, 5)), (5, 9, (5, 9))]

# A-layer blocks 0..5: (target rot idx, ctrl rot idx, target wire, ctrl wire)
A_INFO = []
for _b, (_w1, _w2, (_c, _t)) in enumerate(_BLOCKS[:6]):
    A_INFO.append((2 * _b if _t == _w1 else 2 * _b + 1,
                   2 * _b if _c == _w1 else 2 * _b + 1, _t, _c))

# sin30 layout: 6 groups of [q,r]-pairs over A-blocks [0,3,5,1,2,4]
# (b3 and b2 carry 2 and 4 D's), then 6 ctrl cols.
# D12 = [az6, az7, az7, az8, s9a, s9b, p0a, p0b, p1a, p1b, ua, ub]
_GROUPS = [(0, 2, 0), (2, 4, 3), (6, 2, 5), (8, 4, 1), (12, 8, 2),
           (20, 4, 4)]          # (m30 start, width, A-block)

# smalls column layout
SM_X = 0                 # own-node X angle cols (4)
SM_EA = 4                # e for rel A, per j-block col: [128, 16]
SM_EB = 20
SM_PHI = 36              # phi30
SM_AMP = 66              # amp30
SM_W = 96

# ---------------------------------------------------------------------------
# Host-side circuit-constant preparation
# ---------------------------------------------------------------------------

_PAULI = np.array([
    [[0, 1], [1, 0]],
    [[0, -1j], [1j, 0]],
    [[1, 0], [0, -1]],
], dtype=np.complex128)


def _rot_so3(p):
    """SO(3) Bloch rotation of Rot(phi, theta, omega) = RZ(om) RY(th) RZ(phi)."""
    phi, th, om = float(p[0]), float(p[1]), float(p[2])
    c, s = np.cos(th / 2), np.sin(th / 2)
    U = np.array([
        [np.exp(-0.5j * (phi + om)) * c, -np.exp(0.5j * (phi - om)) * s],
        [np.exp(-0.5j * (phi - om)) * s, np.exp(0.5j * (phi + om)) * c],
    ])
    R = np.empty((3, 3))
    for i in range(3):
        for j in range(3):
            R[i, j] = 0.5 * np.real(
                np.trace(_PAULI[i] @ U @ _PAULI[j] @ U.conj().T))
    return R


def _pack_tables(theta):
    """phi30/amp30 for the amplitude-phase sin tile (see module docstring)."""
    th = np.asarray(theta, np.float64)
    R = [_rot_so3(th[3 * k:3 * k + 3]) for k in range(23)]

    def split_ab(row2, Rt):
        return row2[0] * Rt[0, :], row2[1] * Rt[1, :] + row2[2] * Rt[2, :]

    a_s9, b_s9 = split_ab(R[18][2], R[13])
    v0 = R[20][2, 0] * R[19][0, :]
    v1 = R[20][2, 1] * R[19][1, :] + R[20][2, 2] * R[19][2, :]
    a_p0, b_p0 = split_ab(v0, R[14])
    a_p1, b_p1 = split_ab(v1, R[14])
    a_u, b_u = split_ab(R[21][2], R[16])

    D_order = [(R[12][2], 0), (R[15][2], 3), (R[15][2], 3), (R[17][2], 5),
               (a_s9, 1), (b_s9, 1), (a_p0, 2), (b_p0, 2), (a_p1, 2),
               (b_p1, 2), (a_u, 4), (b_u, 4)]

    phi30 = np.zeros(30)
    amp30 = np.zeros(30)
    for j, (kappa, b) in enumerate(D_order):
        Rt, Rc = R[A_INFO[b][0]], R[A_INFO[b][1]]
        cs, cc = kappa[0] * Rt[0, 0], kappa[0] * Rt[0, 2]
        amp30[2 * j] = np.hypot(cs, cc)
        phi30[2 * j] = np.arctan2(cc, cs)
        cs = kappa[1] * Rt[1, 0] + kappa[2] * Rt[2, 0]
        cc = kappa[1] * Rt[1, 2] + kappa[2] * Rt[2, 2]
        amp30[2 * j + 1] = np.hypot(cs, cc) * np.hypot(Rc[2, 0], Rc[2, 2])
        phi30[2 * j + 1] = np.arctan2(cc, cs)
    for g, b in enumerate([0, 3, 5, 1, 2, 4]):
        Rc = R[A_INFO[b][1]]
        phi30[24 + g] = np.arctan2(Rc[2, 2], Rc[2, 0])
        amp30[24 + g] = 1.0
    return phi30.astype(np.float32), amp30.astype(np.float32)


# ---------------------------------------------------------------------------
# Walrus workaround: this build rejects >1 sync-wait per instruction
# ---------------------------------------------------------------------------


def _split_multi_waits(nc):
    for f in nc.m.functions:
        for bb in f.blocks:
            out = []
            for inst in bb.instructions:
                si = inst.sync_info
                if si is not None and si.on_wait and len(si.on_wait) > 1:
                    waits = list(si.on_wait)
                    for i, w in enumerate(waits[:-1]):
                        out.append(mybir.InstNoOp(
                            name=f"{inst.name}_wsplit{i}",
                            engine=inst.engine,
                            ins=[], outs=[],
                            sync_info=bass_rust.SyncInfo(
                                on_wait=[w], on_update=[]),
                        ))
                    inst.sync_info = bass_rust.SyncInfo(
                        on_wait=[waits[-1]], on_update=list(si.on_update))
                out.append(inst)
            bb.instructions = out


# ---------------------------------------------------------------------------
# Device kernel
# ---------------------------------------------------------------------------


def _build_nc():
    nc = bass.Bass("TRN2", target_bir_lowering=False, num_devices=N_CORES)

    roa_d = {r: nc.declare_dram_parameter(f"roa_{r}", [P, EPAD], F8,
                                          isOutput=False) for r in "ab"}
    rit_d = {r: nc.declare_dram_parameter(f"rit_{r}", [P, EPAD], F8,
                                          isOutput=False) for r in "ab"}
    xs_d = nc.declare_dram_parameter("xs", [P, NCH * 8], BF16, isOutput=False)
    sm_d = nc.declare_dram_parameter("smalls", [P, SM_W], F32, isOutput=False)
    out_d = nc.declare_dram_parameter("out", [P, 1], F32, isOutput=True)

    MUL = mybir.AluOpType.mult
    ADD = mybir.AluOpType.add

    with tile.TileContext(nc) as tc:
        import contextlib
        scope = (tc.spectator_scope if _SCOPES else
                 (lambda name: contextlib.nullcontext()))
        with (
            tc.tile_pool(name="sb", bufs=1) as sb,
            tc.tile_pool(name="s1p", bufs=4, space="PSUM") as s1p,
            tc.tile_pool(name="accp", bufs=2, space="PSUM") as accp,
        ):
            sc = scope("ld")
            sc.__enter__()
            roa_sb, rit_sb = {}, {}
            for r in "ab":
                roa_sb[r] = sb.tile([P, EPAD], F8, name=f"roa_{r}")
                rit_sb[r] = sb.tile([P, EPAD], F8, name=f"rit_{r}")
            nc.sync.dma_start(roa_sb["a"][:, 0:P], roa_d["a"][:, 0:P])
            xs_sb = sb.tile([P, NCH * 8], BF16, name="xs_sb")
            nc.sync.dma_start(xs_sb[:], xs_d[:])
            nc.sync.dma_start(roa_sb["a"][:, P:], roa_d["a"][:, P:])
            sm_sb = sb.tile([P, SM_W], F32, name="sm_sb")
            nc.sync.dma_start(sm_sb[:], sm_d[:])
            nc.sync.dma_start(roa_sb["b"][:], roa_d["b"][:])
            for r in "ab":
                nc.gpsimd.dma_start(rit_sb[r][:], rit_d[r][:])

            # preload the ACT Sin table while DMAs stream; warm the PE
            warm = sb.tile([P, 8], F32, name="warm")
            nc.vector.memset(warm[:], 0.0)
            nc.scalar.activation(warm[:, 0:1], warm[:, 0:1],
                                 mybir.ActivationFunctionType.Sin)
            warm16 = sb.tile([P, P], BF16, name="warm16")
            nc.vector.memset(warm16[:], 0.0)
            for i in range(3):
                wp = s1p.tile([P, 8], F32, name=f"warm_ps{i}", tag="s1")
                nc.tensor.matmul(wp[:], warm16[:], warm16[:, 0:8],
                                 start=True, stop=True)
            sc.__exit__(None, None, None)

            # ---- stage 1 + 2 per relation --------------------------------
            sc = scope("s1")
            sc.__enter__()
            ENG = [nc.vector, nc.gpsimd]
            bhl = {}
            for ri, r in enumerate("ab"):
                beo = sb.tile([P, NB * D], F32, name=f"beo_{r}")
                eoff = SM_EA if ri == 0 else SM_EB
                for m in range(NB):
                    ps = s1p.tile([P, D], F32, name=f"s1ps_{r}{m}", tag="s1")
                    c8 = (m // 2) * 8
                    nc.tensor.matmul(
                        ps[:], roa_sb[r][:, m * P:(m + 1) * P],
                        xs_sb[:, c8:c8 + 4], start=True, stop=False)
                    nc.tensor.matmul(
                        ps[:], roa_sb[r][:, m * P:(m + 1) * P],
                        xs_sb[:, c8 + 4:c8 + 8], start=False, stop=True)
                    if m % 2 == 0:
                        nc.vector.tensor_scalar(
                            beo[:, m * D:(m + 1) * D], ps[:],
                            sm_sb[:, eoff + m:eoff + m + 1], None, MUL)
                    else:
                        nc.scalar.mul(
                            beo[:, m * D:(m + 1) * D], ps[:],
                            sm_sb[:, eoff + m:eoff + m + 1])
                # split into bf16 hi/lo pairs [hi(4) | lo(4)] per j-block
                hl = sb.tile([P, NB * 8], BF16, name=f"bhl_{r}")
                hl3 = hl.rearrange("p (m c) -> p m c", c=8)
                hi_view, lo_view = hl3[:, :, 0:4], hl3[:, :, 4:8]
                beo3 = beo.rearrange("p (m d) -> p m d", d=D)
                brs = sb.tile([P, NB * D], F32, name=f"brs_{r}")
                brs3 = brs.rearrange("p (m d) -> p m d", d=D)
                nc.gpsimd.tensor_copy(hi_view, beo3)
                nc.vector.scalar_tensor_tensor(
                    brs3, hi_view, -1.0, beo3, MUL, ADD)
                nc.gpsimd.tensor_copy(lo_view, brs3)
                bhl[r] = hl
            sc.__exit__(None, None, None)

            # ---- stage 3: scatter to own nodes ---------------------------
            sc = scope("s3")
            sc.__enter__()
            mm = sb.tile([P, 8], F32, name="mm")
            for ri, r in enumerate("ab"):
                acc = accp.tile([P, 8], F32, name=f"acc_{r}", tag=f"acc{r}")
                for m in range(NB):
                    nc.tensor.matmul(
                        acc[:], rit_sb[r][:, m * P:(m + 1) * P],
                        bhl[r][:, m * 8:(m + 1) * 8],
                        start=(m == 0), stop=(m == NB - 1))
                tmp = sb.tile([P, 4], F32, name=f"s3ev_{r}", tag="ev",
                              bufs=4)
                nc.vector.tensor_copy(tmp[:], acc[:, 0:4])
                nc.vector.tensor_tensor(
                    mm[:, ri * 4:(ri + 1) * 4], tmp[:], acc[:, 4:8], ADD)
            sc.__exit__(None, None, None)

            # ---- circuit ------------------------------------------------
            sc = scope("ci")
            sc.__enter__()

            def phi(s, w):
                return sm_sb[:, SM_PHI + s:SM_PHI + s + w]

            # m30 build: per group, angle col broadcast + phi
            m30 = sb.tile([P, 30], F32, name="m30")
            # AP-scalar (Ptr) ops only run on DVE; ACT takes the bias form
            BENG = [nc.vector, nc.scalar]
            # target angle source per group: wires [1, 6, 10, 2, 5, 9]
            # -> [mi1, mo2, X2, mi2, mo1, X1]
            tsrc = [mm[:, 1:2], mm[:, 6:7], sm_sb[:, SM_X + 2:SM_X + 3],
                    mm[:, 2:3], mm[:, 5:6], sm_sb[:, SM_X + 1:SM_X + 2]]
            for g, (s, w, _b) in enumerate(_GROUPS):
                eng = BENG[g % 2]
                if eng is nc.scalar:
                    eng.add(m30[:, s:s + w], phi(s, w), tsrc[g])
                else:
                    eng.tensor_scalar(m30[:, s:s + w], phi(s, w), tsrc[g],
                                      None, ADD)
            # ctrl cols 24:30 <- wires [0, 7, 11, 3, 4, 8]
            csrc = [mm[:, 0:1], mm[:, 7:8], sm_sb[:, SM_X + 3:SM_X + 4],
                    mm[:, 3:4], mm[:, 4:5], sm_sb[:, SM_X:SM_X + 1]]
            for g in range(6):
                s = 24 + g
                eng = BENG[g % 2]
                if eng is nc.scalar:
                    eng.add(m30[:, s:s + 1], phi(s, 1), csrc[g])
                else:
                    eng.tensor_scalar(m30[:, s:s + 1], phi(s, 1), csrc[g],
                                      None, ADD)

            # wrap into [-pi, pi] (|m+phi| < 5*pi so two wraps suffice)
            if _FUSED_DVE:
                nc.vector.add_range_wrap(m30[:], m30[:], 0.0, PI, 2 * PI)
                nc.vector.add_range_wrap(m30[:], m30[:], 0.0, PI, 2 * PI)
            else:
                # rne(m/2pi) via f32->i32 cast (RNE on the DVE), then
                # m - 2pi*k, clamped
                t_f = sb.tile([P, 30], F32, name="t_f")
                t_i = sb.tile([P, 30], mybir.dt.int32, name="t_i")
                t_r = sb.tile([P, 30], F32, name="t_r")
                nc.vector.tensor_scalar(
                    t_f[:], m30[:], float(1.0 / (2 * PI)), None, MUL)
                nc.vector.tensor_copy(t_i[:], t_f[:])
                nc.vector.tensor_copy(t_r[:], t_i[:])
                nc.vector.scalar_tensor_tensor(
                    m30[:], t_r[:], -2 * PI, m30[:], MUL, ADD)
                nc.vector.tensor_scalar(
                    m30[:], m30[:], PI, -PI,
                    mybir.AluOpType.min, mybir.AluOpType.max)

            s30 = sb.tile([P, 30], F32, name="s30")
            nc.scalar.activation(s30[:], m30[:],
                                 mybir.ActivationFunctionType.Sin)
            nc.vector.tensor_tensor(
                s30[:], s30[:], sm_sb[:, SM_AMP:SM_AMP + 30], MUL)

            # D12 = q + az * r per group
            d12 = sb.tile([P, 12], F32, name="d12")
            dpos = 0
            for g, (s, w, _b) in enumerate(_GROUPS):
                nd = w // 2
                if _FUSED_DVE:
                    nc.vector.affine_then_add(
                        d12[:, dpos:dpos + nd],
                        s30[:, s + 1:s + w:2], s30[:, s:s + w:2],
                        s30[:, 24 + g:24 + g + 1], 0.0)
                else:
                    nc.vector.tensor_scalar(
                        d12[:, dpos:dpos + nd], s30[:, s + 1:s + w:2],
                        s30[:, 24 + g:24 + g + 1], None, MUL)
                    ENG[g % 2].tensor_tensor(
                        d12[:, dpos:dpos + nd], d12[:, dpos:dpos + nd],
                        s30[:, s:s + w:2], ADD)
                dpos += nd

            # F = [s9, p0, p1, u]; z9 = (p0 + s9*p1)*u
            f4 = sb.tile([P, 8], F32, name="f4")
            nc.vector.tensor_tensor(f4[:, 4:8], d12[:, 5:12:2], d12[:, 0:4],
                                    MUL)
            nc.vector.tensor_tensor(f4[:, 0:4], d12[:, 4:12:2], f4[:, 4:8],
                                    ADD)
            t2 = sb.tile([P, 2], F32, name="t2")
            nc.vector.tensor_tensor(t2[:, 0:1], f4[:, 0:1], f4[:, 2:3], MUL)
            nc.vector.tensor_tensor(t2[:, 1:2], f4[:, 1:2], t2[:, 0:1], ADD)
            res = sb.tile([P, 1], F32, name="res")
            nc.vector.tensor_tensor(res[:], t2[:, 1:2], f4[:, 3:4], MUL)
            nc.vector.tensor_scalar(res[:], res[:], -PI, PI, MUL, ADD)
            nc.gpsimd.dma_start(out_d[:], res[:])
            sc.__exit__(None, None, None)

    return nc


_NC_CACHE = {}
_RUN_KWARGS = {}      # test harness can set e.g. {"trace": True}
_LAST_RESULTS = []    # BassKernelResults of the most recent run


def _get_nc():
    if "nc" not in _NC_CACHE:
        nc = _build_nc()
        _split_multi_waits(nc)
        _NC_CACHE["nc"] = nc
    return _NC_CACHE["nc"]


def _shard_rel(idx_t, idx_s, e):
    """Per-core (roa, rita, e64) for one relation.

    Core k owns edges with idx_t in its node slice, sorted by source
    chunk; runs padded to RPAD. roa[src%128, j] = 1 (stage-1 stationary,
    partition = node-in-chunk), rita[j%128, (j//128)*128 + tgt%128] = 1
    (stage-3 stationary, partition = j-in-block, host p-major layout).
    """
    f8 = ml_dtypes.float8_e4m3fn
    outs = []
    for k in range(N_CORES):
        ed = np.where(idx_t // P == k)[0]
        sc = idx_s[ed] // P
        order = np.argsort(sc, kind="stable")
        ed, sc = ed[order], sc[order]
        counts = np.bincount(sc, minlength=NCH)
        if counts.max() > RPAD:
            raise ValueError(f"source-chunk run {counts.max()} > RPAD")
        starts = np.searchsorted(sc, np.arange(NCH))
        j = np.arange(len(ed)) - starts[sc] + sc * RPAD
        roa = np.zeros((P, EPAD), np.float32)
        roa[idx_s[ed] % P, j] = 1.0
        rita = np.zeros((P, EPAD), np.float32)
        rita[j % P, (j // P) * P + idx_t[ed] % P] = 1.0
        e16 = np.zeros((P, NB), np.float32)
        e16[j % P, j // P] = e[ed]
        outs.append((roa.astype(f8), rita.astype(f8), e16))
    return outs


def kernel(X, e, Ri, Ro, theta):
    X = np.ascontiguousarray(np.asarray(X, np.float32))
    e = np.ascontiguousarray(np.asarray(e, np.float32))
    Ri = np.asarray(Ri, np.float32)
    Ro = np.asarray(Ro, np.float32)
    theta = np.asarray(theta, np.float32)

    bf = ml_dtypes.bfloat16
    idx_i = np.argmax(Ri, axis=0)
    idx_o = np.argmax(Ro, axis=0)

    # rel A feeds mi (targets idx_i, sources idx_o); rel B feeds mo
    sh_a = _shard_rel(idx_i, idx_o, e)
    sh_b = _shard_rel(idx_o, idx_i, e)

    xh = X.astype(bf).astype(np.float32)
    xl = X - xh
    xs = np.zeros((P, NCH, 8), np.float32)
    xs[:, :, 0:4] = xh.reshape(NCH, P, D).transpose(1, 0, 2)
    xs[:, :, 4:8] = xl.reshape(NCH, P, D).transpose(1, 0, 2)
    xs = np.ascontiguousarray(xs.reshape(P, NCH * 8).astype(bf))

    phi30, amp30 = _pack_tables(theta)

    in_maps = []
    for k in range(N_CORES):
        sm = np.zeros((P, SM_W), np.float32)
        sm[:, SM_X:SM_X + 4] = X[k * P:(k + 1) * P]
        sm[:, SM_EA:SM_EA + NB] = sh_a[k][2]
        sm[:, SM_EB:SM_EB + NB] = sh_b[k][2]
        sm[:, SM_PHI:SM_PHI + 30] = phi30[None, :]
        sm[:, SM_AMP:SM_AMP + 30] = amp30[None, :]
        in_maps.append({
            "roa_a": np.ascontiguousarray(sh_a[k][0]),
            "rit_a": np.ascontiguousarray(sh_a[k][1]),
            "roa_b": np.ascontiguousarray(sh_b[k][0]),
            "rit_b": np.ascontiguousarray(sh_b[k][1]),
            "xs": xs,
            "smalls": np.ascontiguousarray(sm),
        })

    nc = _get_nc()
    res = run_bass_kernel_spmd(nc, in_maps, core_ids=list(range(N_CORES)),
                               **_RUN_KWARGS)
    _LAST_RESULTS.clear()
    _LAST_RESULTS.append(res)
    return np.concatenate(
        [res.results[k]["out"].reshape(-1) for k in range(N_CORES)]
    ).astype(np.float32)


# revision 25
# speedup vs baseline: 3.3471x; 1.0118x over previous
"""Trainium2 Bass kernel for nn_NodeNet: GNN message passing + 12-qubit TTN.

Collective-free sharding: the host owns the edge partition, so core k
receives exactly the edges whose TARGET node lands in its 128-node
slice - once for mi (targets = idx_i) and once for mo (targets =
idx_o). Both contractions are then fully local:

  stage 1 (gather):  beo[j] = e_j * X[src_j]   as a matmul against the
      one-hot source matrix, chunked 128x128 so each chunk's stationary
      is the fp8 0/1 block (exact) and the moving operand is X hi/lo
      bf16 [128, 8]. Edges are sorted by source chunk; per-chunk runs
      are padded to RPAD=256 (max real run ~170).
  stage 2: evict hi+lo, scale by e (f32), re-split to bf16 hi/lo.
  stage 3 (scatter): mi[n] = sum_j beo[j] * RiT[j, n] with stationary =
      the one-hot target block [128, 128] fp8 and moving = beo hi/lo
      [128, 8]; PSUM accumulates over the 16 j-blocks and the output is
      node-partitioned [128, 8] directly - no transposes, no collective.

The circuit contracts to per-node Bloch chains (CNOT target keeps
(x, az*y, az*z); measurement is <Z_9>). Every linear term collapses to
A*sin(m + phi) with host-precomputed amplitude/phase, so the whole
chain is ~30 wide vector ops: build 30 angle columns, wrap to [-pi,pi],
one Sin activation, amplitude multiply, 6 fused (q + az*r) ops, and a
5-op tail.
"""

import ml_dtypes
import numpy as np

import bass_rust
import concourse.bass as bass
import concourse.mybir as mybir
import concourse.tile as tile
from concourse.bass_utils import run_bass_kernel_spmd

F32 = mybir.dt.float32
BF16 = mybir.dt.bfloat16
F8 = mybir.dt.float8e4
N_CORES = 8
N, E, D = 1024, 8192, 4
P = 128                  # partitions / nodes per core
NCH = N // P             # 8 node chunks
RPAD = 256               # padded edges per source chunk (max real ~170)
EPAD = NCH * RPAD        # 2048 padded edges per core per relation
NB = EPAD // P           # 16 j-blocks
PI = float(np.pi)

_SCOPES = False          # test harness can flip on for phase attribution
_FUSED_WRAP = False  # custom-DVE InstISA broken in this walrus build
_FUSED_AFFINE = False       # use custom-DVE fused ops (add_range_wrap etc.)

_BLOCKS = [(0, 1, (0, 1)), (2, 3, (3, 2)), (4, 5, (4, 5)), (6, 7, (7, 6)),
           (8, 9, (8, 9)), (10, 11, (11, 10)), (1, 2, (1, 2)), (5, 6, (6, 5)),
           (9, 10, (10, 9)), (2, 5, (2, 5)), (5, 9, (5, 9))]

# A-layer blocks 0..5: (target rot idx, ctrl rot idx, target wire, ctrl wire)
A_INFO = []
for _b, (_w1, _w2, (_c, _t)) in enumerate(_BLOCKS[:6]):
    A_INFO.append((2 * _b if _t == _w1 else 2 * _b + 1,
                   2 * _b if _c == _w1 else 2 * _b + 1, _t, _c))

# sin30 layout: 6 groups of [q,r]-pairs over A-blocks [0,3,5,1,2,4]
# (b3 and b2 carry 2 and 4 D's), then 6 ctrl cols.
# D12 = [az6, az7, az7, az8, s9a, s9b, p0a, p0b, p1a, p1b, ua, ub]
_GROUPS = [(0, 2, 0), (2, 4, 3), (6, 2, 5), (8, 4, 1), (12, 8, 2),
           (20, 4, 4)]          # (m30 start, width, A-block)

# smalls column layout
SM_X = 0                 # own-node X angle cols (4)
SM_EA = 4                # e for rel A, per j-block col: [128, 16]
SM_EB = 20
SM_PHI = 36              # phi30
SM_AMP = 66              # amp30
SM_W = 96

# ---------------------------------------------------------------------------
# Host-side circuit-constant preparation
# ---------------------------------------------------------------------------

_PAULI = np.array([
    [[0, 1], [1, 0]],
    [[0, -1j], [1j, 0]],
    [[1, 0], [0, -1]],
], dtype=np.complex128)


def _rot_so3(p):
    """SO(3) Bloch rotation of Rot(phi, theta, omega) = RZ(om) RY(th) RZ(phi)."""
    phi, th, om = float(p[0]), float(p[1]), float(p[2])
    c, s = np.cos(th / 2), np.sin(th / 2)
    U = np.array([
        [np.exp(-0.5j * (phi + om)) * c, -np.exp(0.5j * (phi - om)) * s],
        [np.exp(-0.5j * (phi - om)) * s, np.exp(0.5j * (phi + om)) * c],
    ])
    R = np.empty((3, 3))
    for i in range(3):
        for j in range(3):
            R[i, j] = 0.5 * np.real(
                np.trace(_PAULI[i] @ U @ _PAULI[j] @ U.conj().T))
    return R


def _pack_tables(theta):
    """phi30/amp30 for the amplitude-phase sin tile (see module docstring)."""
    th = np.asarray(theta, np.float64)
    R = [_rot_so3(th[3 * k:3 * k + 3]) for k in range(23)]

    def split_ab(row2, Rt):
        return row2[0] * Rt[0, :], row2[1] * Rt[1, :] + row2[2] * Rt[2, :]

    a_s9, b_s9 = split_ab(R[18][2], R[13])
    v0 = R[20][2, 0] * R[19][0, :]
    v1 = R[20][2, 1] * R[19][1, :] + R[20][2, 2] * R[19][2, :]
    a_p0, b_p0 = split_ab(v0, R[14])
    a_p1, b_p1 = split_ab(v1, R[14])
    a_u, b_u = split_ab(R[21][2], R[16])

    D_order = [(R[12][2], 0), (R[15][2], 3), (R[15][2], 3), (R[17][2], 5),
               (a_s9, 1), (b_s9, 1), (a_p0, 2), (b_p0, 2), (a_p1, 2),
               (b_p1, 2), (a_u, 4), (b_u, 4)]

    phi30 = np.zeros(30)
    amp30 = np.zeros(30)
    for j, (kappa, b) in enumerate(D_order):
        Rt, Rc = R[A_INFO[b][0]], R[A_INFO[b][1]]
        cs, cc = kappa[0] * Rt[0, 0], kappa[0] * Rt[0, 2]
        amp30[2 * j] = np.hypot(cs, cc)
        phi30[2 * j] = np.arctan2(cc, cs)
        cs = kappa[1] * Rt[1, 0] + kappa[2] * Rt[2, 0]
        cc = kappa[1] * Rt[1, 2] + kappa[2] * Rt[2, 2]
        amp30[2 * j + 1] = np.hypot(cs, cc) * np.hypot(Rc[2, 0], Rc[2, 2])
        phi30[2 * j + 1] = np.arctan2(cc, cs)
    for g, b in enumerate([0, 3, 5, 1, 2, 4]):
        Rc = R[A_INFO[b][1]]
        phi30[24 + g] = np.arctan2(Rc[2, 2], Rc[2, 0])
        amp30[24 + g] = 1.0
    return phi30.astype(np.float32), amp30.astype(np.float32)


# ---------------------------------------------------------------------------
# Walrus workaround: this build rejects >1 sync-wait per instruction
# ---------------------------------------------------------------------------


def _split_multi_waits(nc):
    for f in nc.m.functions:
        for bb in f.blocks:
            out = []
            for inst in bb.instructions:
                si = inst.sync_info
                if si is not None and si.on_wait and len(si.on_wait) > 1:
                    waits = list(si.on_wait)
                    for i, w in enumerate(waits[:-1]):
                        out.append(mybir.InstNoOp(
                            name=f"{inst.name}_wsplit{i}",
                            engine=inst.engine,
                            ins=[], outs=[],
                            sync_info=bass_rust.SyncInfo(
                                on_wait=[w], on_update=[]),
                        ))
                    inst.sync_info = bass_rust.SyncInfo(
                        on_wait=[waits[-1]], on_update=list(si.on_update))
                out.append(inst)
            bb.instructions = out


# ---------------------------------------------------------------------------
# Device kernel
# ---------------------------------------------------------------------------


def _build_nc():
    nc = bass.Bass("TRN2", target_bir_lowering=False, num_devices=N_CORES)

    roa_d = {r: nc.declare_dram_parameter(f"roa_{r}", [P, EPAD], F8,
                                          isOutput=False) for r in "ab"}
    rit_d = {r: nc.declare_dram_parameter(f"rit_{r}", [P, EPAD], F8,
                                          isOutput=False) for r in "ab"}
    xs_d = nc.declare_dram_parameter("xs", [P, NCH * 8], BF16, isOutput=False)
    sm_d = nc.declare_dram_parameter("smalls", [P, SM_W], F32, isOutput=False)
    out_d = nc.declare_dram_parameter("out", [P, 1], F32, isOutput=True)

    MUL = mybir.AluOpType.mult
    ADD = mybir.AluOpType.add

    with tile.TileContext(nc) as tc:
        import contextlib
        scope = (tc.spectator_scope if _SCOPES else
                 (lambda name: contextlib.nullcontext()))
        with (
            tc.tile_pool(name="sb", bufs=1) as sb,
            tc.tile_pool(name="s1p", bufs=4, space="PSUM") as s1p,
            tc.tile_pool(name="accp", bufs=2, space="PSUM") as accp,
        ):
            sc = scope("ld")
            sc.__enter__()
            roa_sb, rit_sb = {}, {}
            for r in "ab":
                roa_sb[r] = sb.tile([P, EPAD], F8, name=f"roa_{r}")
                rit_sb[r] = sb.tile([P, EPAD], F8, name=f"rit_{r}")
            nc.sync.dma_start(roa_sb["a"][:, 0:P], roa_d["a"][:, 0:P])
            xs_sb = sb.tile([P, NCH * 8], BF16, name="xs_sb")
            nc.sync.dma_start(xs_sb[:], xs_d[:])
            nc.sync.dma_start(roa_sb["a"][:, P:], roa_d["a"][:, P:])
            sm_sb = sb.tile([P, SM_W], F32, name="sm_sb")
            nc.sync.dma_start(sm_sb[:], sm_d[:])
            nc.sync.dma_start(roa_sb["b"][:], roa_d["b"][:])
            for r in "ab":
                nc.gpsimd.dma_start(rit_sb[r][:], rit_d[r][:])

            # preload the ACT Sin table while DMAs stream; warm the PE
            warm = sb.tile([P, 8], F32, name="warm")
            nc.vector.memset(warm[:], 0.0)
            nc.scalar.activation(warm[:, 0:1], warm[:, 0:1],
                                 mybir.ActivationFunctionType.Sin)
            warm16 = sb.tile([P, P], BF16, name="warm16")
            nc.vector.memset(warm16[:], 0.0)
            for i in range(3):
                wp = s1p.tile([P, 8], F32, name=f"warm_ps{i}", tag="s1")
                nc.tensor.matmul(wp[:], warm16[:], warm16[:, 0:8],
                                 start=True, stop=True)
            sc.__exit__(None, None, None)

            # ---- stage 1 + 2 per relation --------------------------------
            sc = scope("s1")
            sc.__enter__()
            ENG = [nc.vector, nc.gpsimd]
            bhl = {}
            for ri, r in enumerate("ab"):
                beo = sb.tile([P, NB * D], F32, name=f"beo_{r}")
                eoff = SM_EA if ri == 0 else SM_EB
                for m in range(NB):
                    ps = s1p.tile([P, D], F32, name=f"s1ps_{r}{m}", tag="s1")
                    c8 = (m // 2) * 8
                    nc.tensor.matmul(
                        ps[:], roa_sb[r][:, m * P:(m + 1) * P],
                        xs_sb[:, c8:c8 + 4], start=True, stop=False)
                    nc.tensor.matmul(
                        ps[:], roa_sb[r][:, m * P:(m + 1) * P],
                        xs_sb[:, c8 + 4:c8 + 8], start=False, stop=True)
                    if m % 2 == 0:
                        nc.vector.tensor_scalar(
                            beo[:, m * D:(m + 1) * D], ps[:],
                            sm_sb[:, eoff + m:eoff + m + 1], None, MUL)
                    else:
                        nc.scalar.mul(
                            beo[:, m * D:(m + 1) * D], ps[:],
                            sm_sb[:, eoff + m:eoff + m + 1])
                # split into bf16 hi/lo pairs [hi(4) | lo(4)] per j-block
                hl = sb.tile([P, NB * 8], BF16, name=f"bhl_{r}")
                hl3 = hl.rearrange("p (m c) -> p m c", c=8)
                hi_view, lo_view = hl3[:, :, 0:4], hl3[:, :, 4:8]
                beo3 = beo.rearrange("p (m d) -> p m d", d=D)
                brs = sb.tile([P, NB * D], F32, name=f"brs_{r}")
                brs3 = brs.rearrange("p (m d) -> p m d", d=D)
                nc.gpsimd.tensor_copy(hi_view, beo3)
                nc.vector.scalar_tensor_tensor(
                    brs3, hi_view, -1.0, beo3, MUL, ADD)
                nc.gpsimd.tensor_copy(lo_view, brs3)
                bhl[r] = hl
            sc.__exit__(None, None, None)

            # ---- stage 3: scatter to own nodes ---------------------------
            sc = scope("s3")
            sc.__enter__()
            mm = sb.tile([P, 8], F32, name="mm")
            for ri, r in enumerate("ab"):
                acc = accp.tile([P, 8], F32, name=f"acc_{r}", tag=f"acc{r}")
                for m in range(NB):
                    nc.tensor.matmul(
                        acc[:], rit_sb[r][:, m * P:(m + 1) * P],
                        bhl[r][:, m * 8:(m + 1) * 8],
                        start=(m == 0), stop=(m == NB - 1))
                tmp = sb.tile([P, 4], F32, name=f"s3ev_{r}", tag="ev",
                              bufs=4)
                nc.vector.tensor_copy(tmp[:], acc[:, 0:4])
                nc.vector.tensor_tensor(
                    mm[:, ri * 4:(ri + 1) * 4], tmp[:], acc[:, 4:8], ADD)
            sc.__exit__(None, None, None)

            # ---- circuit ------------------------------------------------
            sc = scope("ci")
            sc.__enter__()

            def phi(s, w):
                return sm_sb[:, SM_PHI + s:SM_PHI + s + w]

            # m30 build: per group, angle col broadcast + phi
            m30 = sb.tile([P, 30], F32, name="m30")
            # AP-scalar (Ptr) ops only run on DVE; ACT takes the bias form
            BENG = [nc.vector, nc.scalar]
            # target angle source per group: wires [1, 6, 10, 2, 5, 9]
            # -> [mi1, mo2, X2, mi2, mo1, X1]
            tsrc = [mm[:, 1:2], mm[:, 6:7], sm_sb[:, SM_X + 2:SM_X + 3],
                    mm[:, 2:3], mm[:, 5:6], sm_sb[:, SM_X + 1:SM_X + 2]]
            for g, (s, w, _b) in enumerate(_GROUPS):
                eng = BENG[g % 2]
                if eng is nc.scalar:
                    eng.add(m30[:, s:s + w], phi(s, w), tsrc[g])
                else:
                    eng.tensor_scalar(m30[:, s:s + w], phi(s, w), tsrc[g],
                                      None, ADD)
            # ctrl cols 24:30 <- wires [0, 7, 11, 3, 4, 8]
            csrc = [mm[:, 0:1], mm[:, 7:8], sm_sb[:, SM_X + 3:SM_X + 4],
                    mm[:, 3:4], mm[:, 4:5], sm_sb[:, SM_X:SM_X + 1]]
            for g in range(6):
                s = 24 + g
                eng = BENG[g % 2]
                if eng is nc.scalar:
                    eng.add(m30[:, s:s + 1], phi(s, 1), csrc[g])
                else:
                    eng.tensor_scalar(m30[:, s:s + 1], phi(s, 1), csrc[g],
                                      None, ADD)

            # wrap into [-pi, pi] (|m+phi| < 5*pi so two wraps suffice)
            if _FUSED_WRAP:
                nc.vector.add_range_wrap(m30[:], m30[:], 0.0, PI, 2 * PI)
                nc.vector.add_range_wrap(m30[:], m30[:], 0.0, PI, 2 * PI)
            else:
                # rne(m/2pi) via f32->i32 cast (RNE on the DVE), then
                # m - 2pi*k, clamped
                t_f = sb.tile([P, 30], F32, name="t_f")
                t_i = sb.tile([P, 30], mybir.dt.int32, name="t_i")
                t_r = sb.tile([P, 30], F32, name="t_r")
                nc.vector.tensor_scalar(
                    t_f[:], m30[:], float(1.0 / (2 * PI)), None, MUL)
                nc.vector.tensor_copy(t_i[:], t_f[:])
                nc.vector.tensor_copy(t_r[:], t_i[:])
                nc.vector.scalar_tensor_tensor(
                    m30[:], t_r[:], -2 * PI, m30[:], MUL, ADD)
                nc.vector.tensor_scalar(
                    m30[:], m30[:], PI, -PI,
                    mybir.AluOpType.min, mybir.AluOpType.max)

            s30 = sb.tile([P, 30], F32, name="s30")
            nc.scalar.activation(s30[:], m30[:],
                                 mybir.ActivationFunctionType.Sin)
            nc.vector.tensor_tensor(
                s30[:], s30[:], sm_sb[:, SM_AMP:SM_AMP + 30], MUL)

            # D12 = q + az * r per group
            d12 = sb.tile([P, 12], F32, name="d12")
            dpos = 0
            for g, (s, w, _b) in enumerate(_GROUPS):
                nd = w // 2
                if _FUSED_AFFINE:
                    nc.vector.affine_then_add(
                        d12[:, dpos:dpos + nd],
                        s30[:, s + 1:s + w:2], s30[:, s:s + w:2],
                        s30[:, 24 + g:24 + g + 1], 0.0)
                else:
                    nc.vector.tensor_scalar(
                        d12[:, dpos:dpos + nd], s30[:, s + 1:s + w:2],
                        s30[:, 24 + g:24 + g + 1], None, MUL)
                    ENG[g % 2].tensor_tensor(
                        d12[:, dpos:dpos + nd], d12[:, dpos:dpos + nd],
                        s30[:, s:s + w:2], ADD)
                dpos += nd

            # F = [s9, p0, p1, u]; z9 = (p0 + s9*p1)*u
            f4 = sb.tile([P, 8], F32, name="f4")
            nc.vector.tensor_tensor(f4[:, 4:8], d12[:, 5:12:2], d12[:, 0:4],
                                    MUL)
            nc.vector.tensor_tensor(f4[:, 0:4], d12[:, 4:12:2], f4[:, 4:8],
                                    ADD)
            t2 = sb.tile([P, 2], F32, name="t2")
            nc.vector.tensor_tensor(t2[:, 0:1], f4[:, 0:1], f4[:, 2:3], MUL)
            nc.vector.tensor_tensor(t2[:, 1:2], f4[:, 1:2], t2[:, 0:1], ADD)
            res = sb.tile([P, 1], F32, name="res")
            nc.vector.tensor_tensor(res[:], t2[:, 1:2], f4[:, 3:4], MUL)
            nc.vector.tensor_scalar(res[:], res[:], -PI, PI, MUL, ADD)
            nc.gpsimd.dma_start(out_d[:], res[:])
            sc.__exit__(None, None, None)

    return nc


_NC_CACHE = {}
_RUN_KWARGS = {}      # test harness can set e.g. {"trace": True}
_LAST_RESULTS = []    # BassKernelResults of the most recent run


def _get_nc():
    if "nc" not in _NC_CACHE:
        nc = _build_nc()
        _split_multi_waits(nc)
        _NC_CACHE["nc"] = nc
    return _NC_CACHE["nc"]


def _shard_rel(idx_t, idx_s, e):
    """Per-core (roa, rita, e64) for one relation.

    Core k owns edges with idx_t in its node slice, sorted by source
    chunk; runs padded to RPAD. roa[src%128, j] = 1 (stage-1 stationary,
    partition = node-in-chunk), rita[j%128, (j//128)*128 + tgt%128] = 1
    (stage-3 stationary, partition = j-in-block, host p-major layout).
    """
    f8 = ml_dtypes.float8_e4m3fn
    outs = []
    for k in range(N_CORES):
        ed = np.where(idx_t // P == k)[0]
        sc = idx_s[ed] // P
        order = np.argsort(sc, kind="stable")
        ed, sc = ed[order], sc[order]
        counts = np.bincount(sc, minlength=NCH)
        if counts.max() > RPAD:
            raise ValueError(f"source-chunk run {counts.max()} > RPAD")
        starts = np.searchsorted(sc, np.arange(NCH))
        j = np.arange(len(ed)) - starts[sc] + sc * RPAD
        roa = np.zeros((P, EPAD), np.float32)
        roa[idx_s[ed] % P, j] = 1.0
        rita = np.zeros((P, EPAD), np.float32)
        rita[j % P, (j // P) * P + idx_t[ed] % P] = 1.0
        e16 = np.zeros((P, NB), np.float32)
        e16[j % P, j // P] = e[ed]
        outs.append((roa.astype(f8), rita.astype(f8), e16))
    return outs


def kernel(X, e, Ri, Ro, theta):
    X = np.ascontiguousarray(np.asarray(X, np.float32))
    e = np.ascontiguousarray(np.asarray(e, np.float32))
    Ri = np.asarray(Ri, np.float32)
    Ro = np.asarray(Ro, np.float32)
    theta = np.asarray(theta, np.float32)

    bf = ml_dtypes.bfloat16
    idx_i = np.argmax(Ri, axis=0)
    idx_o = np.argmax(Ro, axis=0)

    # rel A feeds mi (targets idx_i, sources idx_o); rel B feeds mo
    sh_a = _shard_rel(idx_i, idx_o, e)
    sh_b = _shard_rel(idx_o, idx_i, e)

    xh = X.astype(bf).astype(np.float32)
    xl = X - xh
    xs = np.zeros((P, NCH, 8), np.float32)
    xs[:, :, 0:4] = xh.reshape(NCH, P, D).transpose(1, 0, 2)
    xs[:, :, 4:8] = xl.reshape(NCH, P, D).transpose(1, 0, 2)
    xs = np.ascontiguousarray(xs.reshape(P, NCH * 8).astype(bf))

    phi30, amp30 = _pack_tables(theta)

    in_maps = []
    for k in range(N_CORES):
        sm = np.zeros((P, SM_W), np.float32)
        sm[:, SM_X:SM_X + 4] = X[k * P:(k + 1) * P]
        sm[:, SM_EA:SM_EA + NB] = sh_a[k][2]
        sm[:, SM_EB:SM_EB + NB] = sh_b[k][2]
        sm[:, SM_PHI:SM_PHI + 30] = phi30[None, :]
        sm[:, SM_AMP:SM_AMP + 30] = amp30[None, :]
        in_maps.append({
            "roa_a": np.ascontiguousarray(sh_a[k][0]),
            "rit_a": np.ascontiguousarray(sh_a[k][1]),
            "roa_b": np.ascontiguousarray(sh_b[k][0]),
            "rit_b": np.ascontiguousarray(sh_b[k][1]),
            "xs": xs,
            "smalls": np.ascontiguousarray(sm),
        })

    nc = _get_nc()
    res = run_bass_kernel_spmd(nc, in_maps, core_ids=list(range(N_CORES)),
                               **_RUN_KWARGS)
    _LAST_RESULTS.clear()
    _LAST_RESULTS.append(res)
    return np.concatenate(
        [res.results[k]["out"].reshape(-1) for k in range(N_CORES)]
    ).astype(np.float32)
